# revision 12
# baseline (speedup 1.0000x reference)
"""Trainium2 Bass kernel for nn_DEINA: encoder + Koopman linear recurrence.

Self-contained: shards the batch (512 trajectories) over 8 NeuronCores
(64 trajectories each), runs a fused encoder + blocked-scan recurrence
per core, and gathers the full outputs.

Math (per trajectory, T=256 steps, D=64, H=256, G=192, L=256):
    g  = relu(x Wx1 + bx1); g = relu(g Wx2 + bx2); g = g Wx3
    y  = [x, g]                                  (output 1)
    v  = relu(u Wu1 + bu1) Wu2;  uu = [u, v];  Bu = uu WB
    y_pred[0] = y[0];  y_pred[t+1] = y_pred[t] K + bK + Bu[t]   (output 2)

v3 = v1 blocked-scan recurrence + two structural wins:
  - u-path folded: Bu = u WB[:64] + relu(u Wu1 + bu1) (Wu2 WB[64:]),
    with Wt = Wu2 WB[64:] precomputed on device. No uu materialization.
  - K-power chain in f32r (4x faster than fp32 on the PE).

The time recurrence is parallelized as a blocked scan with S=16:
    phase 1: per block b, z[b,0]=0; z[b,j+1] = z[b,j] K + c[b*S+j]
             (batched over all blocks -> wide matmuls)
    phase 2: y_start[b+1] = y_start[b] K^S + z[b,S]   (16 tiny serial steps)
    phase 3: y_pred[b*S+j] = y_start[b] K^j + z[b,j]  (parallel over b,j;
             emitted directly in natural [token, L] layout)

Precision: encoder in bf16 (fp32 accumulate), recurrence in f32r
(~13-bit mantissa, full PE rate) with K powers chained in f32r.
"""

import numpy as np

import concourse.bacc as bacc
import concourse.bass as bass
import concourse.tile as tile
from concourse import mybir
from concourse.bass import ts
from concourse.bass_utils import run_bass_kernel_spmd
from concourse.masks import make_identity

F32 = mybir.dt.float32
F32R = mybir.dt.float32r
BF16 = mybir.dt.bfloat16
RELU = mybir.ActivationFunctionType.Relu

NCORES = 8
BL = 64  # trajectories per core
T = 256
D = 64
H = 256
G = 192
L = 256
S = 16  # scan block size (= time steps per chunk)
NB = 16  # number of blocks
NG = 4  # block groups
GB = 4  # blocks per group
NTOK = S * BL  # tokens per block wave


def _build():
    nc = bacc.Bacc("TRN2", target_bir_lowering=False)

    x_h = nc.dram_tensor("x", [BL, T, D], F32, kind="ExternalInput")
    u_h = nc.dram_tensor("u", [BL, T, D], F32, kind="ExternalInput")
    wx1_h = nc.dram_tensor("Wx1", [D, H], F32, kind="ExternalInput")
    bx1_h = nc.dram_tensor("bx1", [H], F32, kind="ExternalInput")
    wx2_h = nc.dram_tensor("Wx2", [H, H], F32, kind="ExternalInput")
    bx2_h = nc.dram_tensor("bx2", [H], F32, kind="ExternalInput")
    wx3_h = nc.dram_tensor("Wx3", [H, G], F32, kind="ExternalInput")
    wu1_h = nc.dram_tensor("Wu1", [D, H], F32, kind="ExternalInput")
    bu1_h = nc.dram_tensor("bu1", [H], F32, kind="ExternalInput")
    wu2_h = nc.dram_tensor("Wu2", [H, G], F32, kind="ExternalInput")
    wb_h = nc.dram_tensor("WB", [L, L], F32, kind="ExternalInput")
    wk_h = nc.dram_tensor("WK", [L, L], F32, kind="ExternalInput")
    bk_h = nc.dram_tensor("bK", [L], F32, kind="ExternalInput")
    y_h = nc.dram_tensor("y", [BL, T, L], F32, kind="ExternalOutput")
    yp_h = nc.dram_tensor("y_pred", [BL, T, L], F32, kind="ExternalOutput")

    with tile.TileContext(nc) as tc, tile.ExitStack() as ctx:
        wpool = ctx.enter_context(tc.tile_pool(name="w", bufs=1))
        encpool = ctx.enter_context(tc.tile_pool(name="enc", bufs=3))
        actpool = ctx.enter_context(tc.tile_pool(name="act", bufs=2))
        czpool = ctx.enter_context(tc.tile_pool(name="cz", bufs=2))
        yspool = ctx.enter_context(tc.tile_pool(name="ys", bufs=2))
        yppool = ctx.enter_context(tc.tile_pool(name="ypd", bufs=3))
        pwpool = ctx.enter_context(tc.tile_pool(name="pw", bufs=2))
        encps = ctx.enter_context(tc.tile_pool(name="encps", bufs=2, space="PSUM"))
        tpps = ctx.enter_context(tc.tile_pool(name="tpps", bufs=1, space="PSUM"))
        sps = ctx.enter_context(tc.tile_pool(name="sps", bufs=3, space="PSUM"))

        # ------------------------------------------------------------------
        # Weights / constants
        # ------------------------------------------------------------------
        def load_f32(ap, shape, name):
            t = wpool.tile(shape, F32, tag=name, name=name)
            nc.sync.dma_start(t[:], ap)
            return t

        def to_bf16(src, name):
            t = wpool.tile(list(src.shape), BF16, tag=name, name=name)
            nc.vector.tensor_copy(t[:], src[:])
            return t

        # L1 weights: wx1 lives on partitions 0:64, wu1 on 64:128
        wx1f = wpool.tile([D, H], F32, tag="wx1f")
        nc.sync.dma_start(wx1f[:], wx1_h[:, :])
        wx1b = to_bf16(wx1f, "wx1b")
        wu1f = wpool.tile([128, H], F32, tag="wu1f")
        nc.sync.dma_start(wu1f[64:128, :], wu1_h[:, :])
        wu1b = wpool.tile([128, H], BF16, tag="wu1b")
        nc.vector.tensor_copy(wu1b[64:128, :], wu1f[64:128, :])

        # WB rows 0:64 on partitions 64:128 (lhsT for the K=64 Bu part)
        wbuf = wpool.tile([128, L], F32, tag="wbuf")
        nc.sync.dma_start(wbuf[64:128, :], wb_h[0:64, :])
        wbub = wpool.tile([128, L], BF16, tag="wbub")
        nc.vector.tensor_copy(wbub[64:128, :], wbuf[64:128, :])

        wx2b, wx3b = [], []
        for lt in range(2):
            wx2b.append(to_bf16(load_f32(wx2_h.ap()[ts(lt, 128), :], [128, H], f"wx2f{lt}"), f"wx2b{lt}"))
            wx3b.append(to_bf16(load_f32(wx3_h.ap()[ts(lt, 128), :], [128, G], f"wx3f{lt}"), f"wx3b{lt}"))

        # for Wt = Wu2 @ WB[64:]
        wu2f = [load_f32(wu2_h.ap()[ts(lt, 128), :], [128, G], f"wu2f{lt}") for lt in range(2)]
        wblf0 = load_f32(wb_h.ap()[64:192, :], [128, L], "wblf0")
        wblf1 = load_f32(wb_h.ap()[192:256, :], [64, L], "wblf1")

        # biases as per-partition scalars: col j holds b[j*128 + p]
        def load_bias(h, name):
            t = wpool.tile([128, 2], F32, tag=name, name=name)
            nc.sync.dma_start(t[:], h.rearrange("(t p) -> p t", p=128))
            return t

        bx1v = load_bias(bx1_h, "bx1v")
        bx2v = load_bias(bx2_h, "bx2v")
        bu1v = load_bias(bu1_h, "bu1v")
        bkv = load_bias(bk_h, "bkv")

        # K tiles (f32) and identities
        kf = [load_f32(wk_h.ap()[ts(lt, 128), :], [128, L], f"kf{lt}") for lt in range(2)]
        ident = wpool.tile([128, 128], F32, tag="ident")
        make_identity(nc, ident[:])
        # identity64 on partitions 0:64 (for the input PE transposes)
        identb64 = wpool.tile([64, 64], BF16, tag="identb64")
        nc.gpsimd.dma_start(identb64[:], ident[0:64, 0:64])

        # K^T tiles (for the power chain): kT[b][p, a] = K[a, b*128+p]
        kT = [wpool.tile([128, L], F32, tag=f"kT{lt}", name=f"kT{lt}") for lt in range(2)]
        for a in range(2):
            for b in range(2):
                pst = sps.tile([128, 128], F32, tag="sps", name="pstT_t")
                nc.tensor.transpose(pst[:], kf[a][:, ts(b, 128)], ident[:])
                nc.scalar.copy(kT[b][:, ts(a, 128)], pst[:])

        # Wu2^T: g-tile0 [128, 256h], g-tile1 [64, 256h]
        wu2T0 = wpool.tile([128, H], F32, tag="wu2T0")
        wu2T1 = wpool.tile([64, H], F32, tag="wu2T1")
        for lt in range(2):
            pst = sps.tile([128, 128], F32, tag="sps", name="wu2T_t")
            nc.tensor.transpose(pst[:], wu2f[lt][:, 0:128], ident[:])
            nc.scalar.copy(wu2T0[:, ts(lt, 128)], pst[:])
            pst = sps.tile([128, 128], F32, tag="sps", name="wu2T_t")
            nc.tensor.transpose(pst[0:64, :], wu2f[lt][:, 128:192], ident[:])
            nc.scalar.copy(wu2T1[:, ts(lt, 128)], pst[0:64, :])

        # Wt = Wu2 @ WB[64:]  -> [H, L] bf16, 2 h-tiles
        wtb = []
        for ht in range(2):
            ps = sps.tile([128, L], F32, tag="sps", name="wt_t")
            nc.tensor.matmul(ps[:], wu2T0[:, ts(ht, 128)], wblf0[:], start=True, stop=False)
            nc.tensor.matmul(ps[:], wu2T1[:, ts(ht, 128)], wblf1[:], start=False, stop=True)
            t = wpool.tile([128, L], BF16, tag=f"wtb{ht}", name=f"wtb{ht}")
            nc.any.tensor_copy(t[:], ps[:])
            wtb.append(t)

        # K powers P_j = K^j (natural layout), j = 1..16, f32r chain.
        kTr = []
        for bt in range(2):
            t = wpool.tile([128, L], F32R, tag=f"kTr{bt}")
            nc.vector.tensor_copy(t[:], kT[bt][:])
            kTr.append(t)
        pr = {}  # (j, lt) -> f32r tile
        for lt in range(2):
            pr[(1, lt)] = wpool.tile([128, L], F32R, tag=f"pr1_{lt}", name=f"pr1_{lt}")
            nc.vector.tensor_copy(pr[(1, lt)][:], kf[lt][:])
        pf_prev = [pr[(1, lt)] for lt in range(2)]
        for j in range(2, S + 1):
            psts = [sps.tile([128, L], F32, tag="sps", name="pstP_t") for _ in range(2)]
            for bt in range(2):
                for rt in range(2):
                    nc.tensor.matmul(
                        psts[rt][:],
                        kTr[bt][:, ts(rt, 128)],
                        pf_prev[bt][:],
                        start=(bt == 0),
                        stop=(bt == 1),
                    )
            pf_cur = []
            for rt in range(2):
                pr[(j, rt)] = wpool.tile([128, L], F32R, tag=f"pr{j}_{rt}", name=f"pr{j}_{rt}")
                nc.any.tensor_copy(pr[(j, rt)][:], psts[rt][:])
                pf_cur.append(pr[(j, rt)])
            pf_prev = pf_cur

        # ------------------------------------------------------------------
        # Views for strided HBM I/O
        # ------------------------------------------------------------------
        # g-part of y: rows (j2, traj), free (mt, l); t = b*16 + mt*2 + j2
        yv_g = y_h.rearrange("traj (b mt j2) l -> b j2 traj mt l", b=NB, mt=8, j2=2)
        # y_pred: rows (nb2, traj), free (mt, l) for fixed (group, j)
        ypv = yp_h.rearrange(
            "traj (g mt nb2 j) l -> g j nb2 traj mt l", g=NG, mt=2, nb2=2, j=S
        )

        cz = {}  # (group, lt) -> [128, S, GB, BL] f32r tile
        ys = {}  # (group, lt) -> [128, GB, BL] f32r tile

        def get_cz(g, lt):
            if (g, lt) not in cz:
                cz[(g, lt)] = czpool.tile([128, S, GB, BL], F32R, tag=f"cz{lt}", name=f"cz{g}_{lt}")
            return cz[(g, lt)]

        def get_ys(g, lt):
            if (g, lt) not in ys:
                ys[(g, lt)] = yspool.tile([128, GB, BL], F32R, tag=f"ys{lt}", name=f"ys{g}_{lt}")
            return ys[(g, lt)]

        # ------------------------------------------------------------------
        # Encoder chunk: one block b (16 time steps x 64 trajectories),
        # processed as one 1024-token wave
        # ------------------------------------------------------------------
        def encoder_chunk(b):
            g = b // GB
            big = b % GB
            # natural f32 x/u slabs [traj, t, d]; bf16 [x | u] slab built by
            # on-chip casts (x is only read from HBM once)
            xf = encpool.tile([BL, S, D], F32, tag="xf", name="xf")
            nc.gpsimd.dma_start(xf[:], x_h[:, ts(b, S), :])
            uf = encpool.tile([BL, S, D], F32, tag="uf", name="uf")
            nc.gpsimd.dma_start(uf[:], u_h[:, ts(b, S), :])
            sxu = encpool.tile([BL, S, 2 * D], BF16, tag="sxu", name="sxu")
            nc.vector.tensor_copy(sxu[:, :, 0:D], xf[:])
            nc.vector.tensor_copy(sxu[:, :, D : 2 * D], uf[:])

            # PE-transpose the 16 [64,128] t-slabs into one psum bank:
            # partitions (x-d | u-d), cols (t, traj)
            xps = tpps.tile([128, S * BL], BF16, tag="tpps", name="tpps_t")
            for t in range(S):
                nc.tensor.matmul(
                    xps[:, ts(t, BL)], sxu[:, t, :], identb64[:],
                    is_transpose=True, start=(t == 0), stop=(t == S - 1),
                )
            xu = encpool.tile([128, S * BL], BF16, tag="xu", name="xu")
            nc.vector.tensor_copy(xu[:], xps[:])

            # y x-part write (f32 exact)
            nc.sync.dma_start(y_h[:, ts(b, S), 0:D], xf[:])

            rx = xu[0:D, :]
            ru = xu[D:128, :]

            # L1: h1x = relu(Wx1^T x^T + bx1), h1u likewise (K=64)
            h1xs, h1us = [], []
            for mt in range(2):
                psx = encps.tile([128, NTOK], F32, tag="encps", name="encps_t")
                psu = encps.tile([128, NTOK], F32, tag="encps", name="encps_t")
                for hf in range(2):
                    # x on array rows 0:64, u on rows 64:128 -> concurrent
                    nc.tensor.matmul(
                        psx[:, ts(hf, 512)], wx1b[:, ts(mt, 128)], rx[:, ts(hf, 512)],
                        start=True, stop=True,
                    )
                    nc.tensor.matmul(
                        psu[:, ts(hf, 512)], wu1b[64:128, ts(mt, 128)], ru[:, ts(hf, 512)],
                        start=True, stop=True, tile_position=(64, 0),
                    )
                sbx = actpool.tile([128, NTOK], BF16, tag=f"h1x{mt}", name=f"h1x{mt}_t")
                nc.scalar.activation(sbx[:], psx[:], RELU, bias=bx1v[:, mt : mt + 1])
                h1xs.append(sbx)
                sbu = actpool.tile([128, NTOK], BF16, tag=f"h1u{mt}", name=f"h1u{mt}_t")
                nc.vector.tensor_scalar(
                    sbu[:], psu[:], bu1v[:, mt : mt + 1], 0.0,
                    op0=mybir.AluOpType.add, op1=mybir.AluOpType.max,
                )
                h1us.append(sbu)

            # L2: h2x = relu(Wx2^T h1x + bx2)
            # both mt psums live; loops ordered so consecutive matmuls
            # stream the same moving operand (avoids rhs-stream restarts)
            ps2 = [encps.tile([128, NTOK], F32, tag="encps", name="encps_t") for _ in range(2)]
            for lt in range(2):
                for hf in range(2):
                    for mt in range(2):
                        nc.tensor.matmul(
                            ps2[mt][:, ts(hf, 512)], wx2b[lt][:, ts(mt, 128)],
                            h1xs[lt][:, ts(hf, 512)],
                            start=(lt == 0), stop=(lt == 1),
                        )
            h2xs = []
            for mt in range(2):
                sb = actpool.tile([128, NTOK], BF16, tag=f"h2x{mt}", name=f"h2x{mt}_t")
                nc.scalar.activation(sb[:], ps2[mt][:], RELU, bias=bx2v[:, mt : mt + 1])
                h2xs.append(sb)

            # Bu = u WB[:64] + h1u Wt (K=64 part row-packed at (64,0)),
            # c = Bu + bK -> cz (f32r); rhs-reuse ordering across mt
            psb = [encps.tile([128, NTOK], F32, tag="encps", name="encps_t") for _ in range(2)]
            for hf in range(2):
                for mt in range(2):
                    nc.tensor.matmul(
                        psb[mt][:, ts(hf, 512)], wbub[64:128, ts(mt, 128)],
                        ru[:, ts(hf, 512)],
                        start=True, stop=False, tile_position=(64, 0),
                    )
            for lt in range(2):
                for hf in range(2):
                    for mt in range(2):
                        nc.tensor.matmul(
                            psb[mt][:, ts(hf, 512)], wtb[lt][:, ts(mt, 128)],
                            h1us[lt][:, ts(hf, 512)],
                            start=False, stop=(lt == 1),
                        )
            for mt in range(2):
                czt = get_cz(g, mt)
                nc.vector.tensor_scalar_add(
                    czt[:, :, big, :],
                    psb[mt][:].rearrange("p (a c) -> p a c", a=S),
                    bkv[:, mt : mt + 1],
                )

            # g (natural layout) = h2x @ Wx3: 8 M-tiles in 2 psum tiles
            gps = [encps.tile([128, NTOK], F32, tag="encps", name="gps_t") for _ in range(2)]
            for mt8 in range(8):
                out = gps[mt8 // 4][:, (mt8 % 4) * 256 : (mt8 % 4) * 256 + G]
                for lt in range(2):
                    nc.tensor.matmul(
                        out, h2xs[lt][:, ts(mt8, 128)], wx3b[lt][:],
                        start=(lt == 0), stop=(lt == 1),
                    )
            gs = actpool.tile([128, 8, G], F32, tag="gs", name="gs_t")
            for half in range(2):
                nc.any.tensor_copy(
                    gs[:, ts(half, 4), :],
                    gps[half][:].rearrange("p (m x) -> p m x", m=4)[:, :, 0:G],
                )
            for j2 in range(2):
                nc.sync.dma_start(yv_g[b, j2][:, :, D:L], gs[ts(j2, 64), :, :])

            # y0 (t = 0): assemble y_start[0] transposed, fp32 x-part
            if b == 0:
                y0a = sps.tile([128, BL], F32, tag="sps", name="y0a_t")
                nc.tensor.matmul(
                    y0a[0:D, :], xf[:, 0, :], ident[0:D, 0:D],
                    is_transpose=True, start=True, stop=True,
                )
                for lt in range(2):
                    nc.tensor.matmul(
                        y0a[64:128, :], wx3b[lt][:, 0:64], h2xs[lt][:, 0:BL],
                        start=(lt == 0), stop=(lt == 1), tile_position=(0, 64),
                    )
                nc.any.tensor_copy(get_ys(0, 0)[:, 0, :], y0a[:])
                y0b = sps.tile([128, BL], F32, tag="sps", name="y0b_t")
                for lt in range(2):
                    nc.tensor.matmul(
                        y0b[:], wx3b[lt][:, 64:192], h2xs[lt][:, 0:BL],
                        start=(lt == 0), stop=(lt == 1),
                    )
                nc.any.tensor_copy(get_ys(0, 1)[:, 0, :], y0b[:])

        # ------------------------------------------------------------------
        # Phase 1: batched local scans (per group)
        # ------------------------------------------------------------------
        def phase1(g):
            czt = [get_cz(g, lt) for lt in range(2)]
            for j in range(1, S):
                zprev = [czt[lt][:, j - 1, :, :].rearrange("p a c -> p (a c)") for lt in range(2)]
                ps = sps.tile([128, 512], F32, tag="sps", name="p1ps_t")
                for l1t in range(2):
                    for l2t in range(2):
                        nc.tensor.matmul(
                            ps[:, ts(l2t, GB * BL)],
                            pr[(1, l1t)][:, ts(l2t, 128)], zprev[l1t],
                            start=(l1t == 0 and l2t == 0),
                            stop=(l1t == 1 and l2t == 1),
                        )
                for l2t in range(2):
                    nc.vector.tensor_add(
                        czt[l2t][:, j, :, :],
                        ps[:, ts(l2t, GB * BL)].rearrange("p (b c) -> p b c", b=GB),
                        czt[l2t][:, j, :, :],
                    )

        # ------------------------------------------------------------------
        # Phase 2: block-level scan (serial, 4 steps per group)
        # ------------------------------------------------------------------
        def phase2(g):
            for nb in range(GB):
                b = g * GB + nb
                if b == NB - 1:
                    break
                ng, nnb = (g, nb + 1) if nb + 1 < GB else (g + 1, 0)
                ps = sps.tile([128, 2 * BL], F32, tag="sps", name="p2ps_t")
                for l1t in range(2):
                    for lt in range(2):
                        nc.tensor.matmul(
                            ps[:, ts(lt, BL)],
                            pr[(S, l1t)][:, ts(lt, 128)], get_ys(g, l1t)[:, nb, :],
                            start=(l1t == 0 and lt == 0),
                            stop=(l1t == 1 and lt == 1),
                        )
                for lt in range(2):
                    nc.vector.tensor_add(
                        get_ys(ng, lt)[:, nnb, :], ps[:, ts(lt, BL)],
                        get_cz(g, lt)[:, S - 1, nb, :].bitcast(F32),
                    )

        # ------------------------------------------------------------------
        # Phase 3: fix-up, natural-layout output
        # ------------------------------------------------------------------
        def phase3(g):
            for j in range(S):
                ysb = yppool.tile([128, 2, L], F32, tag="ysb", name="ysb_t")
                ps = sps.tile([128, 2 * L], F32, tag="sps", name="p3ps_t")
                if j > 0:
                    for l1t in range(2):
                        for mt in range(2):
                            nc.tensor.matmul(
                                ps[:, ts(mt, L)],
                                get_ys(g, l1t)[:, ts(mt, 2), :].rearrange("p a c -> p (a c)"),
                                pr[(j, l1t)][:],
                                start=(mt == 0 and l1t == 0), stop=False,
                            )
                    for mt in range(2):
                        for lt in range(2):
                            nc.tensor.matmul(
                                ps[:, mt * L + lt * 128 : mt * L + lt * 128 + 128],
                                get_cz(g, lt)[:, j - 1, ts(mt, 2), :].rearrange("p a c -> p (a c)").bitcast(F32),
                                ident[:],
                                is_transpose=True, start=False,
                                stop=(mt == 1 and lt == 1),
                            )
                else:
                    for mt in range(2):
                        for lt in range(2):
                            nc.tensor.matmul(
                                ps[:, mt * L + lt * 128 : mt * L + lt * 128 + 128],
                                get_ys(g, lt)[:, ts(mt, 2), :].rearrange("p a c -> p (a c)").bitcast(F32),
                                ident[:],
                                is_transpose=True, start=(mt == 0 and lt == 0),
                                stop=(mt == 1 and lt == 1),
                            )
                nc.vector.tensor_copy(ysb[:, 0, :], ps[:, 0:L])
                nc.scalar.copy(ysb[:, 1, :], ps[:, L : 2 * L])
                for nb2 in range(2):
                    nc.sync.dma_start(ypv[g, j][nb2], ysb[ts(nb2, 64), :, :])

        # ------------------------------------------------------------------
        # Emit
        # ------------------------------------------------------------------
        for g in range(NG):
            for big in range(GB):
                encoder_chunk(g * GB + big)
            phase1(g)
            phase2(g)
            phase3(g)

    nc.compile()
    return nc


_NC = None


def _get_nc():
    global _NC
    if _NC is None:
        _NC = _build()
    return _NC


def kernel(**inputs):
    nc = _get_nc()
    wnames = [
        "Wx1", "bx1", "Wx2", "bx2", "Wx3", "Wu1", "bu1", "Wu2", "WB", "WK", "bK",
    ]
    weights = {k: np.ascontiguousarray(np.asarray(inputs[k], dtype=np.float32)) for k in wnames}
    x = np.asarray(inputs["x"], dtype=np.float32)
    u = np.asarray(inputs["u"], dtype=np.float32)
    in_maps = []
    for c in range(NCORES):
        m = dict(weights)
        m["x"] = np.ascontiguousarray(x[c * BL : (c + 1) * BL])
        m["u"] = np.ascontiguousarray(u[c * BL : (c + 1) * BL])
        in_maps.append(m)
    res = run_bass_kernel_spmd(nc, in_maps, core_ids=list(range(NCORES)))
    y = np.concatenate([r["y"] for r in res.results], axis=0)
    y_pred = np.concatenate([r["y_pred"] for r in res.results], axis=0)
    return (y, y_pred)


# revision 16
# speedup vs baseline: 1.1773x; 1.1773x over previous
"""Trainium2 Bass kernel for nn_DEINA: encoder + Koopman linear recurrence.

Self-contained: shards the batch (512 trajectories) over 8 NeuronCores
(64 trajectories each), runs a fused encoder + blocked-scan recurrence
per core, and gathers the full outputs.

Math (per trajectory, T=256 steps, D=64, H=256, G=192, L=256):
    g  = relu(x Wx1 + bx1); g = relu(g Wx2 + bx2); g = g Wx3
    y  = [x, g]                                  (output 1)
    v  = relu(u Wu1 + bu1) Wu2;  uu = [u, v];  Bu = uu WB
    y_pred[0] = y[0];  y_pred[t+1] = y_pred[t] K + bK + Bu[t]   (output 2)

v3 = v1 blocked-scan recurrence + two structural wins:
  - u-path folded: Bu = u WB[:64] + relu(u Wu1 + bu1) (Wu2 WB[64:]),
    with Wt = Wu2 WB[64:] precomputed on device. No uu materialization.
  - K-power chain in f32r (4x faster than fp32 on the PE).

The time recurrence is parallelized as a blocked scan with S=16:
    phase 1: per block b, z[b,0]=0; z[b,j+1] = z[b,j] K + c[b*S+j]
             (batched over all blocks -> wide matmuls)
    phase 2: y_start[b+1] = y_start[b] K^S + z[b,S]   (16 tiny serial steps)
    phase 3: y_pred[b*S+j] = y_start[b] K^j + z[b,j]  (parallel over b,j;
             emitted directly in natural [token, L] layout)

Precision: encoder in bf16 (fp32 accumulate), recurrence in f32r
(~13-bit mantissa, full PE rate) with K powers chained in f32r.
"""

import numpy as np

import concourse.bacc as bacc
import concourse.bass as bass
import concourse.tile as tile
from concourse import mybir
from concourse.bass import ts
from concourse.bass_utils import run_bass_kernel_spmd
from concourse.masks import make_identity

F32 = mybir.dt.float32
F32R = mybir.dt.float32r
BF16 = mybir.dt.bfloat16
RELU = mybir.ActivationFunctionType.Relu

NCORES = 8
BL = 64  # trajectories per core
T = 256
D = 64
H = 256
G = 192
L = 256
S = 16  # scan block size (= time steps per chunk)
NB = 16  # number of blocks
NG = 4  # block groups
GB = 4  # blocks per group
NTOK = S * BL  # tokens per block wave


def _build():
    nc = bacc.Bacc("TRN2", target_bir_lowering=False)

    x_h = nc.dram_tensor("x", [BL, T, D], F32, kind="ExternalInput")
    u_h = nc.dram_tensor("u", [BL, T, D], F32, kind="ExternalInput")
    wx1_h = nc.dram_tensor("Wx1", [D, H], F32, kind="ExternalInput")
    bx1_h = nc.dram_tensor("bx1", [H], F32, kind="ExternalInput")
    wx2_h = nc.dram_tensor("Wx2", [H, H], F32, kind="ExternalInput")
    bx2_h = nc.dram_tensor("bx2", [H], F32, kind="ExternalInput")
    wx3_h = nc.dram_tensor("Wx3", [H, G], F32, kind="ExternalInput")
    wu1_h = nc.dram_tensor("Wu1", [D, H], F32, kind="ExternalInput")
    bu1_h = nc.dram_tensor("bu1", [H], F32, kind="ExternalInput")
    wu2_h = nc.dram_tensor("Wu2", [H, G], F32, kind="ExternalInput")
    wb_h = nc.dram_tensor("WB", [L, L], F32, kind="ExternalInput")
    wk_h = nc.dram_tensor("WK", [L, L], F32, kind="ExternalInput")
    bk_h = nc.dram_tensor("bK", [L], F32, kind="ExternalInput")
    y_h = nc.dram_tensor("y", [BL, T, L], F32, kind="ExternalOutput")
    yp_h = nc.dram_tensor("y_pred", [BL, T, L], F32, kind="ExternalOutput")

    with tile.TileContext(nc) as tc, tile.ExitStack() as ctx:
        wpool = ctx.enter_context(tc.tile_pool(name="w", bufs=1))
        encpool = ctx.enter_context(tc.tile_pool(name="enc", bufs=3))
        actpool = ctx.enter_context(tc.tile_pool(name="act", bufs=2))
        czpool = ctx.enter_context(tc.tile_pool(name="cz", bufs=2))
        yspool = ctx.enter_context(tc.tile_pool(name="ys", bufs=2))
        yppool = ctx.enter_context(tc.tile_pool(name="ypd", bufs=3))
        pwpool = ctx.enter_context(tc.tile_pool(name="pw", bufs=2))
        encps = ctx.enter_context(tc.tile_pool(name="encps", bufs=2, space="PSUM"))
        tpps = ctx.enter_context(tc.tile_pool(name="tpps", bufs=1, space="PSUM"))
        sps = ctx.enter_context(tc.tile_pool(name="sps", bufs=3, space="PSUM"))

        # ------------------------------------------------------------------
        # Head fast-path: input slabs for blocks 0-1 lead the sync queue
        # ------------------------------------------------------------------
        fastin = {}
        for b in range(2):
            xf0 = encpool.tile([BL, S, D], F32, tag="xf", name="xf")
            nc.sync.dma_start(xf0[:], x_h[:, ts(b, S), :])
            uf0 = encpool.tile([BL, S, D], F32, tag="uf", name=f"uf{b}")
            nc.sync.dma_start(uf0[:], u_h[:, ts(b, S), :])
            fastin[b] = (xf0, uf0)

        # ------------------------------------------------------------------
        # Weights / constants
        # ------------------------------------------------------------------
        def load_f32(ap, shape, name):
            t = wpool.tile(shape, F32, tag=name, name=name)
            nc.sync.dma_start(t[:], ap)
            return t

        def to_bf16(src, name):
            t = wpool.tile(list(src.shape), BF16, tag=name, name=name)
            nc.vector.tensor_copy(t[:], src[:])
            return t

        # L1 weights: wx1 lives on partitions 0:64, wu1 on 64:128
        wx1f = wpool.tile([D, H], F32, tag="wx1f")
        nc.sync.dma_start(wx1f[:], wx1_h[:, :])
        wx1b = to_bf16(wx1f, "wx1b")
        wu1f = wpool.tile([128, H], F32, tag="wu1f")
        nc.sync.dma_start(wu1f[64:128, :], wu1_h[:, :])
        wu1b = wpool.tile([128, H], BF16, tag="wu1b")
        nc.vector.tensor_copy(wu1b[64:128, :], wu1f[64:128, :])

        # WB rows 0:64 on partitions 64:128 (lhsT for the K=64 Bu part)
        wbuf = wpool.tile([128, L], F32, tag="wbuf")
        nc.sync.dma_start(wbuf[64:128, :], wb_h[0:64, :])
        wbub = wpool.tile([128, L], BF16, tag="wbub")
        nc.vector.tensor_copy(wbub[64:128, :], wbuf[64:128, :])

        wx2b, wx3b = [], []
        for lt in range(2):
            wx2b.append(to_bf16(load_f32(wx2_h.ap()[ts(lt, 128), :], [128, H], f"wx2f{lt}"), f"wx2b{lt}"))
            wx3b.append(to_bf16(load_f32(wx3_h.ap()[ts(lt, 128), :], [128, G], f"wx3f{lt}"), f"wx3b{lt}"))

        # for Wt = Wu2 @ WB[64:]
        wu2f = [load_f32(wu2_h.ap()[ts(lt, 128), :], [128, G], f"wu2f{lt}") for lt in range(2)]
        wblf0 = load_f32(wb_h.ap()[64:192, :], [128, L], "wblf0")
        wblf1 = load_f32(wb_h.ap()[192:256, :], [64, L], "wblf1")

        # biases as per-partition scalars: col j holds b[j*128 + p]
        def load_bias(h, name):
            t = wpool.tile([128, 2], F32, tag=name, name=name)
            nc.sync.dma_start(t[:], h.rearrange("(t p) -> p t", p=128))
            return t

        bx1v = load_bias(bx1_h, "bx1v")
        bx2v = load_bias(bx2_h, "bx2v")
        bu1v = load_bias(bu1_h, "bu1v")
        bkv = load_bias(bk_h, "bkv")

        # K tiles (f32) and identities
        kf = [load_f32(wk_h.ap()[ts(lt, 128), :], [128, L], f"kf{lt}") for lt in range(2)]
        ident = wpool.tile([128, 128], F32, tag="ident")
        make_identity(nc, ident[:])
        # identity64 on partitions 0:64 (for the input PE transposes)
        identb64 = wpool.tile([64, 64], BF16, tag="identb64")
        nc.gpsimd.dma_start(identb64[:], ident[0:64, 0:64])

        # K^T tiles (for the power chain): kT[b][p, a] = K[a, b*128+p]
        kT = [wpool.tile([128, L], F32, tag=f"kT{lt}", name=f"kT{lt}") for lt in range(2)]
        for a in range(2):
            for b in range(2):
                pst = sps.tile([128, 128], F32, tag="sps", name="pstT_t")
                nc.tensor.transpose(pst[:], kf[a][:, ts(b, 128)], ident[:])
                nc.scalar.copy(kT[b][:, ts(a, 128)], pst[:])

        # Wu2^T: g-tile0 [128, 256h], g-tile1 [64, 256h]
        wu2T0 = wpool.tile([128, H], F32, tag="wu2T0")
        wu2T1 = wpool.tile([64, H], F32, tag="wu2T1")
        for lt in range(2):
            pst = sps.tile([128, 128], F32, tag="sps", name="wu2T_t")
            nc.tensor.transpose(pst[:], wu2f[lt][:, 0:128], ident[:])
            nc.scalar.copy(wu2T0[:, ts(lt, 128)], pst[:])
            pst = sps.tile([128, 128], F32, tag="sps", name="wu2T_t")
            nc.tensor.transpose(pst[0:64, :], wu2f[lt][:, 128:192], ident[:])
            nc.scalar.copy(wu2T1[:, ts(lt, 128)], pst[0:64, :])

        # Wt = Wu2 @ WB[64:]  -> [H, L] bf16, 2 h-tiles
        wtb = []
        for ht in range(2):
            ps = sps.tile([128, L], F32, tag="sps", name="wt_t")
            nc.tensor.matmul(ps[:], wu2T0[:, ts(ht, 128)], wblf0[:], start=True, stop=False)
            nc.tensor.matmul(ps[:], wu2T1[:, ts(ht, 128)], wblf1[:], start=False, stop=True)
            t = wpool.tile([128, L], BF16, tag=f"wtb{ht}", name=f"wtb{ht}")
            nc.any.tensor_copy(t[:], ps[:])
            wtb.append(t)

        # K powers P_j = K^j (natural layout), j = 1..16, f32r chain.
        kTr = []
        for bt in range(2):
            t = wpool.tile([128, L], F32R, tag=f"kTr{bt}")
            nc.vector.tensor_copy(t[:], kT[bt][:])
            kTr.append(t)
        pr = {}  # (j, lt) -> f32r tile
        for lt in range(2):
            pr[(1, lt)] = wpool.tile([128, L], F32R, tag=f"pr1_{lt}", name=f"pr1_{lt}")
            nc.vector.tensor_copy(pr[(1, lt)][:], kf[lt][:])
        pf_prev = [pr[(1, lt)] for lt in range(2)]
        for j in range(2, S + 1):
            psts = [sps.tile([128, L], F32, tag="sps", name="pstP_t") for _ in range(2)]
            for bt in range(2):
                for rt in range(2):
                    nc.tensor.matmul(
                        psts[rt][:],
                        kTr[bt][:, ts(rt, 128)],
                        pf_prev[bt][:],
                        start=(bt == 0),
                        stop=(bt == 1),
                    )
            pf_cur = []
            for rt in range(2):
                pr[(j, rt)] = wpool.tile([128, L], F32R, tag=f"pr{j}_{rt}", name=f"pr{j}_{rt}")
                nc.any.tensor_copy(pr[(j, rt)][:], psts[rt][:])
                pf_cur.append(pr[(j, rt)])
            pf_prev = pf_cur

        # ------------------------------------------------------------------
        # Views for strided HBM I/O
        # ------------------------------------------------------------------
        # g-part of y: rows (j2, traj), free (mt, l); t = b*16 + mt*2 + j2
        yv_g = y_h.rearrange("traj (b mt j2) l -> b j2 traj mt l", b=NB, mt=8, j2=2)
        # y_pred: rows (nb2, traj), free (mt, l) for fixed (group, j)
        ypv = yp_h.rearrange(
            "traj (g mt nb2 j) l -> g j nb2 traj mt l", g=NG, mt=2, nb2=2, j=S
        )

        cz = {}  # (group, lt) -> [128, S, GB, BL] f32r tile
        ys = {}  # (group, lt) -> [128, GB, BL] f32r tile

        def get_cz(g, lt):
            if (g, lt) not in cz:
                cz[(g, lt)] = czpool.tile([128, S, GB, BL], F32R, tag=f"cz{lt}", name=f"cz{g}_{lt}")
            return cz[(g, lt)]

        def get_ys(g, lt):
            if (g, lt) not in ys:
                ys[(g, lt)] = yspool.tile([128, GB, BL], F32R, tag=f"ys{lt}", name=f"ys{g}_{lt}")
            return ys[(g, lt)]

        # ------------------------------------------------------------------
        # Encoder chunk: one block b (16 time steps x 64 trajectories),
        # processed as one 1024-token wave
        # ------------------------------------------------------------------
        def encoder_chunk(b):
            g = b // GB
            big = b % GB
            # natural f32 x slab [traj, t, d] (feeds y x-part and, for b=0, y0)
            # blocks 0-1 use the idle sync queue + vector casts (fast head);
            # later blocks use gpsimd cast-DMAs (keeps vector queue free)
            if b < 2:
                xf, uf = fastin[b]
                sxu = encpool.tile([BL, S, 2 * D], BF16, tag="sxu", name="sxu")
                nc.vector.tensor_copy(sxu[:, :, 0:D], xf[:])
                nc.vector.tensor_copy(sxu[:, :, D : 2 * D], uf[:])
            else:
                xf = encpool.tile([BL, S, D], F32, tag="xf", name="xf")
                nc.gpsimd.dma_start(xf[:], x_h[:, ts(b, S), :])
                sxu = encpool.tile([BL, S, 2 * D], BF16, tag="sxu", name="sxu")
                nc.gpsimd.dma_start(sxu[:, :, 0:D], x_h[:, ts(b, S), :])
                nc.gpsimd.dma_start(sxu[:, :, D : 2 * D], u_h[:, ts(b, S), :])

            # PE-transpose the 16 [64,128] t-slabs into one psum bank:
            # partitions (x-d | u-d), cols (t, traj)
            xps = tpps.tile([128, S * BL], BF16, tag="tpps", name="tpps_t")
            for t in range(S):
                nc.tensor.matmul(
                    xps[:, ts(t, BL)], sxu[:, t, :], identb64[:],
                    is_transpose=True, start=(t == 0), stop=(t == S - 1),
                )
            xu = encpool.tile([128, S * BL], BF16, tag="xu", name="xu")
            nc.any.tensor_copy(xu[:], xps[:])

            # y x-part write (f32 exact)
            nc.sync.dma_start(y_h[:, ts(b, S), 0:D], xf[:])

            rx = xu[0:D, :]
            ru = xu[D:128, :]

            # L1: h1x = relu(Wx1^T x^T + bx1), h1u likewise (K=64)
            h1xs, h1us = [], []
            for mt in range(2):
                psx = encps.tile([128, NTOK], F32, tag="encps", name="encps_t")
                psu = encps.tile([128, NTOK], F32, tag="encps", name="encps_t")
                for hf in range(2):
                    # x on array rows 0:64, u on rows 64:128 -> concurrent
                    nc.tensor.matmul(
                        psx[:, ts(hf, 512)], wx1b[:, ts(mt, 128)], rx[:, ts(hf, 512)],
                        start=True, stop=True,
                    )
                    nc.tensor.matmul(
                        psu[:, ts(hf, 512)], wu1b[64:128, ts(mt, 128)], ru[:, ts(hf, 512)],
                        start=True, stop=True, tile_position=(64, 0),
                    )
                sbx = actpool.tile([128, NTOK], BF16, tag=f"h1x{mt}", name=f"h1x{mt}_t")
                nc.scalar.activation(sbx[:], psx[:], RELU, bias=bx1v[:, mt : mt + 1])
                h1xs.append(sbx)
                sbu = actpool.tile([128, NTOK], BF16, tag=f"h1u{mt}", name=f"h1u{mt}_t")
                nc.vector.tensor_scalar(
                    sbu[:], psu[:], bu1v[:, mt : mt + 1], 0.0,
                    op0=mybir.AluOpType.add, op1=mybir.AluOpType.max,
                )
                h1us.append(sbu)

            # L2: h2x = relu(Wx2^T h1x + bx2)
            # both mt psums live; loops ordered so consecutive matmuls
            # stream the same moving operand (avoids rhs-stream restarts)
            ps2 = [encps.tile([128, NTOK], F32, tag="encps", name="encps_t") for _ in range(2)]
            for lt in range(2):
                for hf in range(2):
                    for mt in range(2):
                        nc.tensor.matmul(
                            ps2[mt][:, ts(hf, 512)], wx2b[lt][:, ts(mt, 128)],
                            h1xs[lt][:, ts(hf, 512)],
                            start=(lt == 0), stop=(lt == 1),
                        )
            h2xs = []
            for mt in range(2):
                sb = actpool.tile([128, NTOK], BF16, tag=f"h2x{mt}", name=f"h2x{mt}_t")
                nc.scalar.activation(sb[:], ps2[mt][:], RELU, bias=bx2v[:, mt : mt + 1])
                h2xs.append(sb)

            # Bu = u WB[:64] + h1u Wt (K=64 part row-packed at (64,0)),
            # c = Bu + bK -> cz (f32r); rhs-reuse ordering across mt
            psb = [encps.tile([128, NTOK], F32, tag="encps", name="encps_t") for _ in range(2)]
            for hf in range(2):
                for mt in range(2):
                    nc.tensor.matmul(
                        psb[mt][:, ts(hf, 512)], wbub[64:128, ts(mt, 128)],
                        ru[:, ts(hf, 512)],
                        start=True, stop=False, tile_position=(64, 0),
                    )
            for lt in range(2):
                for hf in range(2):
                    for mt in range(2):
                        nc.tensor.matmul(
                            psb[mt][:, ts(hf, 512)], wtb[lt][:, ts(mt, 128)],
                            h1us[lt][:, ts(hf, 512)],
                            start=False, stop=(lt == 1),
                        )
            for mt in range(2):
                czt = get_cz(g, mt)
                nc.vector.tensor_scalar_add(
                    czt[:, :, big, :],
                    psb[mt][:].rearrange("p (a c) -> p a c", a=S),
                    bkv[:, mt : mt + 1],
                )

            # g (natural layout) = h2x @ Wx3: 8 M-tiles in 2 psum tiles
            gps = [encps.tile([128, NTOK], F32, tag="encps", name="gps_t") for _ in range(2)]
            for mt8 in range(8):
                out = gps[mt8 // 4][:, (mt8 % 4) * 256 : (mt8 % 4) * 256 + G]
                for lt in range(2):
                    nc.tensor.matmul(
                        out, h2xs[lt][:, ts(mt8, 128)], wx3b[lt][:],
                        start=(lt == 0), stop=(lt == 1),
                    )
            gs = actpool.tile([128, 8, G], F32, tag="gs", name="gs_t")
            for half in range(2):
                nc.any.tensor_copy(
                    gs[:, ts(half, 4), :],
                    gps[half][:].rearrange("p (m x) -> p m x", m=4)[:, :, 0:G],
                )
            for j2 in range(2):
                nc.sync.dma_start(yv_g[b, j2][:, :, D:L], gs[ts(j2, 64), :, :])

            # y0 (t = 0): assemble y_start[0] transposed, fp32 x-part
            if b == 0:
                y0a = sps.tile([128, BL], F32, tag="sps", name="y0a_t")
                nc.tensor.matmul(
                    y0a[0:D, :], xf[:, 0, :], ident[0:D, 0:D],
                    is_transpose=True, start=True, stop=True,
                )
                for lt in range(2):
                    nc.tensor.matmul(
                        y0a[64:128, :], wx3b[lt][:, 0:64], h2xs[lt][:, 0:BL],
                        start=(lt == 0), stop=(lt == 1), tile_position=(0, 64),
                    )
                nc.any.tensor_copy(get_ys(0, 0)[:, 0, :], y0a[:])
                y0b = sps.tile([128, BL], F32, tag="sps", name="y0b_t")
                for lt in range(2):
                    nc.tensor.matmul(
                        y0b[:], wx3b[lt][:, 64:192], h2xs[lt][:, 0:BL],
                        start=(lt == 0), stop=(lt == 1),
                    )
                nc.any.tensor_copy(get_ys(0, 1)[:, 0, :], y0b[:])

        # ------------------------------------------------------------------
        # Phase 1: batched local scans (per group)
        # ------------------------------------------------------------------
        def phase1(g):
            czt = [get_cz(g, lt) for lt in range(2)]
            for j in range(1, S):
                zprev = [czt[lt][:, j - 1, :, :].rearrange("p a c -> p (a c)") for lt in range(2)]
                ps = sps.tile([128, 512], F32, tag="sps", name="p1ps_t")
                for l1t in range(2):
                    for l2t in range(2):
                        nc.tensor.matmul(
                            ps[:, ts(l2t, GB * BL)],
                            pr[(1, l1t)][:, ts(l2t, 128)], zprev[l1t],
                            start=(l1t == 0 and l2t == 0),
                            stop=(l1t == 1 and l2t == 1),
                        )
                for l2t in range(2):
                    nc.vector.tensor_add(
                        czt[l2t][:, j, :, :],
                        ps[:, ts(l2t, GB * BL)].rearrange("p (b c) -> p b c", b=GB),
                        czt[l2t][:, j, :, :],
                    )

        # ------------------------------------------------------------------
        # Phase 2: block-level scan (serial, 4 steps per group)
        # ------------------------------------------------------------------
        def phase2(g):
            for nb in range(GB):
                b = g * GB + nb
                if b == NB - 1:
                    break
                ng, nnb = (g, nb + 1) if nb + 1 < GB else (g + 1, 0)
                ps = sps.tile([128, 2 * BL], F32, tag="sps", name="p2ps_t")
                for l1t in range(2):
                    for lt in range(2):
                        nc.tensor.matmul(
                            ps[:, ts(lt, BL)],
                            pr[(S, l1t)][:, ts(lt, 128)], get_ys(g, l1t)[:, nb, :],
                            start=(l1t == 0 and lt == 0),
                            stop=(l1t == 1 and lt == 1),
                        )
                for lt in range(2):
                    nc.vector.tensor_add(
                        get_ys(ng, lt)[:, nnb, :], ps[:, ts(lt, BL)],
                        get_cz(g, lt)[:, S - 1, nb, :].bitcast(F32),
                    )

        # ------------------------------------------------------------------
        # Phase 3: fix-up, natural-layout output
        # ------------------------------------------------------------------
        def phase3(g):
            for j in range(S):
                ysb = yppool.tile([128, 2, L], F32, tag="ysb", name="ysb_t")
                ps = sps.tile([128, 2 * L], F32, tag="sps", name="p3ps_t")
                if j > 0:
                    for l1t in range(2):
                        for mt in range(2):
                            nc.tensor.matmul(
                                ps[:, ts(mt, L)],
                                get_ys(g, l1t)[:, ts(mt, 2), :].rearrange("p a c -> p (a c)"),
                                pr[(j, l1t)][:],
                                start=(mt == 0 and l1t == 0), stop=False,
                            )
                    for mt in range(2):
                        for lt in range(2):
                            nc.tensor.matmul(
                                ps[:, mt * L + lt * 128 : mt * L + lt * 128 + 128],
                                get_cz(g, lt)[:, j - 1, ts(mt, 2), :].rearrange("p a c -> p (a c)").bitcast(F32),
                                ident[:],
                                is_transpose=True, start=False,
                                stop=(mt == 1 and lt == 1),
                            )
                else:
                    for mt in range(2):
                        for lt in range(2):
                            nc.tensor.matmul(
                                ps[:, mt * L + lt * 128 : mt * L + lt * 128 + 128],
                                get_ys(g, lt)[:, ts(mt, 2), :].rearrange("p a c -> p (a c)").bitcast(F32),
                                ident[:],
                                is_transpose=True, start=(mt == 0 and lt == 0),
                                stop=(mt == 1 and lt == 1),
                            )
                nc.any.tensor_copy(ysb[:], ps[:].rearrange("p (m x) -> p m x", m=2))
                for nb2 in range(2):
                    nc.sync.dma_start(ypv[g, j][nb2], ysb[ts(nb2, 64), :, :])

        # ------------------------------------------------------------------
        # Emit
        # ------------------------------------------------------------------
        for g in range(NG):
            for big in range(GB):
                encoder_chunk(g * GB + big)
            phase1(g)
            phase2(g)
            phase3(g)

    nc.compile()
    return nc


_NC = None


def _get_nc():
    global _NC
    if _NC is None:
        _NC = _build()
    return _NC


def kernel(**inputs):
    nc = _get_nc()
    wnames = [
        "Wx1", "bx1", "Wx2", "bx2", "Wx3", "Wu1", "bu1", "Wu2", "WB", "WK", "bK",
    ]
    weights = {k: np.ascontiguousarray(np.asarray(inputs[k], dtype=np.float32)) for k in wnames}
    x = np.asarray(inputs["x"], dtype=np.float32)
    u = np.asarray(inputs["u"], dtype=np.float32)
    in_maps = []
    for c in range(NCORES):
        m = dict(weights)
        m["x"] = np.ascontiguousarray(x[c * BL : (c + 1) * BL])
        m["u"] = np.ascontiguousarray(u[c * BL : (c + 1) * BL])
        in_maps.append(m)
    res = run_bass_kernel_spmd(nc, in_maps, core_ids=list(range(NCORES)))
    y = np.concatenate([r["y"] for r in res.results], axis=0)
    y_pred = np.concatenate([r["y_pred"] for r in res.results], axis=0)
    return (y, y_pred)


# revision 19
# speedup vs baseline: 1.2020x; 1.0210x over previous
"""Trainium2 Bass kernel for nn_DEINA: encoder + Koopman linear recurrence.

Self-contained: shards the batch (512 trajectories) over 8 NeuronCores
(64 trajectories each), runs a fused encoder + blocked-scan recurrence
per core, and gathers the full outputs.

Math (per trajectory, T=256 steps, D=64, H=256, G=192, L=256):
    g  = relu(x Wx1 + bx1); g = relu(g Wx2 + bx2); g = g Wx3
    y  = [x, g]                                  (output 1)
    v  = relu(u Wu1 + bu1) Wu2;  uu = [u, v];  Bu = uu WB
    y_pred[0] = y[0];  y_pred[t+1] = y_pred[t] K + bK + Bu[t]   (output 2)

v3 = v1 blocked-scan recurrence + two structural wins:
  - u-path folded: Bu = u WB[:64] + relu(u Wu1 + bu1) (Wu2 WB[64:]),
    with Wt = Wu2 WB[64:] precomputed on device. No uu materialization.
  - K-power chain in f32r (4x faster than fp32 on the PE).

The time recurrence is parallelized as a blocked scan with S=16:
    phase 1: per block b, z[b,0]=0; z[b,j+1] = z[b,j] K + c[b*S+j]
             (batched over all blocks -> wide matmuls)
    phase 2: y_start[b+1] = y_start[b] K^S + z[b,S]   (16 tiny serial steps)
    phase 3: y_pred[b*S+j] = y_start[b] K^j + z[b,j]  (parallel over b,j;
             emitted directly in natural [token, L] layout)

Precision: encoder in bf16 (fp32 accumulate), recurrence in f32r
(~13-bit mantissa, full PE rate) with K powers chained in f32r.
"""

import numpy as np

import concourse.bacc as bacc
import concourse.bass as bass
import concourse.tile as tile
from concourse import mybir
from concourse.bass import ts
from concourse.bass_utils import run_bass_kernel_spmd
from concourse.masks import make_identity

F32 = mybir.dt.float32
F32R = mybir.dt.float32r
BF16 = mybir.dt.bfloat16
RELU = mybir.ActivationFunctionType.Relu

NCORES = 8
BL = 64  # trajectories per core
T = 256
D = 64
H = 256
G = 192
L = 256
S = 16  # scan block size (= time steps per chunk)
NB = 16  # number of blocks
NG = 4  # block groups
GB = 4  # blocks per group
NTOK = S * BL  # tokens per block wave


def _build():
    nc = bacc.Bacc("TRN2", target_bir_lowering=False)

    x_h = nc.dram_tensor("x", [BL, T, D], F32, kind="ExternalInput")
    u_h = nc.dram_tensor("u", [BL, T, D], F32, kind="ExternalInput")
    wx1_h = nc.dram_tensor("Wx1", [D, H], F32, kind="ExternalInput")
    bx1_h = nc.dram_tensor("bx1", [H], F32, kind="ExternalInput")
    wx2_h = nc.dram_tensor("Wx2", [H, H], F32, kind="ExternalInput")
    bx2_h = nc.dram_tensor("bx2", [H], F32, kind="ExternalInput")
    wx3_h = nc.dram_tensor("Wx3", [H, G], F32, kind="ExternalInput")
    wu1_h = nc.dram_tensor("Wu1", [D, H], F32, kind="ExternalInput")
    bu1_h = nc.dram_tensor("bu1", [H], F32, kind="ExternalInput")
    wu2_h = nc.dram_tensor("Wu2", [H, G], F32, kind="ExternalInput")
    wb_h = nc.dram_tensor("WB", [L, L], F32, kind="ExternalInput")
    wk_h = nc.dram_tensor("WK", [L, L], F32, kind="ExternalInput")
    bk_h = nc.dram_tensor("bK", [L], F32, kind="ExternalInput")
    y_h = nc.dram_tensor("y", [BL, T, L], F32, kind="ExternalOutput")
    yp_h = nc.dram_tensor("y_pred", [BL, T, L], F32, kind="ExternalOutput")

    with tile.TileContext(nc) as tc, tile.ExitStack() as ctx:
        wpool = ctx.enter_context(tc.tile_pool(name="w", bufs=1))
        encpool = ctx.enter_context(tc.tile_pool(name="enc", bufs=3))
        actpool = ctx.enter_context(tc.tile_pool(name="act", bufs=2))
        czpool = ctx.enter_context(tc.tile_pool(name="cz", bufs=2))
        yspool = ctx.enter_context(tc.tile_pool(name="ys", bufs=2))
        yppool = ctx.enter_context(tc.tile_pool(name="ypd", bufs=3))
        pwpool = ctx.enter_context(tc.tile_pool(name="pw", bufs=2))
        encps = ctx.enter_context(tc.tile_pool(name="encps", bufs=2, space="PSUM"))
        tpps = ctx.enter_context(tc.tile_pool(name="tpps", bufs=1, space="PSUM"))
        sps = ctx.enter_context(tc.tile_pool(name="sps", bufs=3, space="PSUM"))

        # ------------------------------------------------------------------
        # Head fast-path: input slabs for blocks 0-1 lead the sync queue
        # ------------------------------------------------------------------
        fastin = {}
        for b in range(2):
            xf0 = encpool.tile([BL, S, D], F32, tag="xf", name="xf")
            nc.sync.dma_start(xf0[:], x_h[:, ts(b, S), :])
            uf0 = encpool.tile([BL, S, D], F32, tag="uf", name=f"uf{b}")
            nc.sync.dma_start(uf0[:], u_h[:, ts(b, S), :])
            fastin[b] = (xf0, uf0)

        # ------------------------------------------------------------------
        # Weights / constants
        # ------------------------------------------------------------------
        def load_f32(ap, shape, name):
            t = wpool.tile(shape, F32, tag=name, name=name)
            nc.sync.dma_start(t[:], ap)
            return t

        def to_bf16(src, name):
            t = wpool.tile(list(src.shape), BF16, tag=name, name=name)
            nc.vector.tensor_copy(t[:], src[:])
            return t

        # L1 weights: wx1 lives on partitions 0:64, wu1 on 64:128
        wx1f = wpool.tile([D, H], F32, tag="wx1f")
        nc.sync.dma_start(wx1f[:], wx1_h[:, :])
        wx1b = to_bf16(wx1f, "wx1b")
        wu1f = wpool.tile([128, H], F32, tag="wu1f")
        nc.sync.dma_start(wu1f[64:128, :], wu1_h[:, :])
        wu1b = wpool.tile([128, H], BF16, tag="wu1b")
        nc.vector.tensor_copy(wu1b[64:128, :], wu1f[64:128, :])

        # WB rows 0:64 on partitions 64:128 (lhsT for the K=64 Bu part)
        wbuf = wpool.tile([128, L], F32, tag="wbuf")
        nc.sync.dma_start(wbuf[64:128, :], wb_h[0:64, :])
        wbub = wpool.tile([128, L], BF16, tag="wbub")
        nc.vector.tensor_copy(wbub[64:128, :], wbuf[64:128, :])

        wx2b, wx3b = [], []
        for lt in range(2):
            wx2b.append(to_bf16(load_f32(wx2_h.ap()[ts(lt, 128), :], [128, H], f"wx2f{lt}"), f"wx2b{lt}"))
            wx3b.append(to_bf16(load_f32(wx3_h.ap()[ts(lt, 128), :], [128, G], f"wx3f{lt}"), f"wx3b{lt}"))

        # for Wt = Wu2 @ WB[64:]
        wu2f = [load_f32(wu2_h.ap()[ts(lt, 128), :], [128, G], f"wu2f{lt}") for lt in range(2)]
        wblf0 = load_f32(wb_h.ap()[64:192, :], [128, L], "wblf0")
        wblf1 = load_f32(wb_h.ap()[192:256, :], [64, L], "wblf1")

        # biases as per-partition scalars: col j holds b[j*128 + p]
        def load_bias(h, name):
            t = wpool.tile([128, 2], F32, tag=name, name=name)
            nc.sync.dma_start(t[:], h.rearrange("(t p) -> p t", p=128))
            return t

        bx1v = load_bias(bx1_h, "bx1v")
        bx2v = load_bias(bx2_h, "bx2v")
        bu1v = load_bias(bu1_h, "bu1v")
        bkv = load_bias(bk_h, "bkv")

        # K tiles (f32) and identities
        kf = [load_f32(wk_h.ap()[ts(lt, 128), :], [128, L], f"kf{lt}") for lt in range(2)]
        ident = wpool.tile([128, 128], F32, tag="ident")
        make_identity(nc, ident[:])
        # identity64 on partitions 0:64 (for the input PE transposes)
        identb64 = wpool.tile([64, 64], BF16, tag="identb64")
        nc.gpsimd.dma_start(identb64[:], ident[0:64, 0:64])

        # K^T tiles (for the power chain): kT[b][p, a] = K[a, b*128+p]
        kT = [wpool.tile([128, L], F32, tag=f"kT{lt}", name=f"kT{lt}") for lt in range(2)]
        for a in range(2):
            for b in range(2):
                pst = sps.tile([128, 128], F32, tag="sps", name="pstT_t")
                nc.tensor.transpose(pst[:], kf[a][:, ts(b, 128)], ident[:])
                nc.scalar.copy(kT[b][:, ts(a, 128)], pst[:])

        # Wu2^T: g-tile0 [128, 256h], g-tile1 [64, 256h]
        wu2T0 = wpool.tile([128, H], F32, tag="wu2T0")
        wu2T1 = wpool.tile([64, H], F32, tag="wu2T1")
        for lt in range(2):
            pst = sps.tile([128, 128], F32, tag="sps", name="wu2T_t")
            nc.tensor.transpose(pst[:], wu2f[lt][:, 0:128], ident[:])
            nc.scalar.copy(wu2T0[:, ts(lt, 128)], pst[:])
            pst = sps.tile([128, 128], F32, tag="sps", name="wu2T_t")
            nc.tensor.transpose(pst[0:64, :], wu2f[lt][:, 128:192], ident[:])
            nc.scalar.copy(wu2T1[:, ts(lt, 128)], pst[0:64, :])

        # Wt = Wu2 @ WB[64:]  -> [H, L] bf16, 2 h-tiles
        wtb = []
        for ht in range(2):
            ps = sps.tile([128, L], F32, tag="sps", name="wt_t")
            nc.tensor.matmul(ps[:], wu2T0[:, ts(ht, 128)], wblf0[:], start=True, stop=False)
            nc.tensor.matmul(ps[:], wu2T1[:, ts(ht, 128)], wblf1[:], start=False, stop=True)
            t = wpool.tile([128, L], BF16, tag=f"wtb{ht}", name=f"wtb{ht}")
            nc.any.tensor_copy(t[:], ps[:])
            wtb.append(t)

        # K powers P_j = K^j (natural layout), j = 1..16, f32r chain.
        kTr = []
        for bt in range(2):
            t = wpool.tile([128, L], F32R, tag=f"kTr{bt}")
            nc.vector.tensor_copy(t[:], kT[bt][:])
            kTr.append(t)
        pr = {}  # (j, lt) -> f32r tile
        for lt in range(2):
            pr[(1, lt)] = wpool.tile([128, L], F32R, tag=f"pr1_{lt}", name=f"pr1_{lt}")
            nc.vector.tensor_copy(pr[(1, lt)][:], kf[lt][:])
        pf_prev = [pr[(1, lt)] for lt in range(2)]
        for j in range(2, S + 1):
            psts = [sps.tile([128, L], F32, tag="sps", name="pstP_t") for _ in range(2)]
            for bt in range(2):
                for rt in range(2):
                    nc.tensor.matmul(
                        psts[rt][:],
                        kTr[bt][:, ts(rt, 128)],
                        pf_prev[bt][:],
                        start=(bt == 0),
                        stop=(bt == 1),
                    )
            pf_cur = []
            for rt in range(2):
                pr[(j, rt)] = wpool.tile([128, L], F32R, tag=f"pr{j}_{rt}", name=f"pr{j}_{rt}")
                nc.any.tensor_copy(pr[(j, rt)][:], psts[rt][:])
                pf_cur.append(pr[(j, rt)])
            pf_prev = pf_cur

        # ------------------------------------------------------------------
        # Views for strided HBM I/O
        # ------------------------------------------------------------------
        # g-part of y: rows (j2, traj), free (mt, l); t = b*16 + mt*2 + j2
        yv_g = y_h.rearrange("traj (b mt j2) l -> b j2 traj mt l", b=NB, mt=8, j2=2)
        # y_pred: rows (nb2, traj), free (mt, l) for fixed (group, j)
        ypv = yp_h.rearrange(
            "traj (g mt nb2 j) l -> g j nb2 traj mt l", g=NG, mt=2, nb2=2, j=S
        )

        cz = {}  # (group, lt) -> [128, S, GB, BL] f32r tile
        ys = {}  # (group, lt) -> [128, GB, BL] f32r tile

        def get_cz(g, lt):
            if (g, lt) not in cz:
                cz[(g, lt)] = czpool.tile([128, S, GB, BL], F32R, tag=f"cz{lt}", name=f"cz{g}_{lt}")
            return cz[(g, lt)]

        def get_ys(g, lt):
            if (g, lt) not in ys:
                ys[(g, lt)] = yspool.tile([128, GB, BL], F32R, tag=f"ys{lt}", name=f"ys{g}_{lt}")
            return ys[(g, lt)]

        # ------------------------------------------------------------------
        # Encoder chunk: one block b (16 time steps x 64 trajectories),
        # processed as one 1024-token wave
        # ------------------------------------------------------------------
        def encoder_chunk(b):
            g = b // GB
            big = b % GB
            # natural f32 x slab [traj, t, d] (feeds y x-part and, for b=0, y0)
            # blocks 0-1 use the idle sync queue + vector casts (fast head);
            # later blocks use gpsimd cast-DMAs (keeps vector queue free)
            if b < 2:
                xf, uf = fastin[b]
                sxu = encpool.tile([BL, S, 2 * D], BF16, tag="sxu", name="sxu")
                nc.vector.tensor_copy(sxu[:, :, 0:D], xf[:])
                nc.vector.tensor_copy(sxu[:, :, D : 2 * D], uf[:])
            else:
                xf = encpool.tile([BL, S, D], F32, tag="xf", name="xf")
                nc.gpsimd.dma_start(xf[:], x_h[:, ts(b, S), :])
                sxu = encpool.tile([BL, S, 2 * D], BF16, tag="sxu", name="sxu")
                nc.gpsimd.dma_start(sxu[:, :, 0:D], x_h[:, ts(b, S), :])
                nc.gpsimd.dma_start(sxu[:, :, D : 2 * D], u_h[:, ts(b, S), :])

            # PE-transpose the 16 [64,128] t-slabs into one psum bank:
            # partitions (x-d | u-d), cols (t, traj)
            xps = tpps.tile([128, S * BL], BF16, tag="tpps", name="tpps_t")
            for t in range(S):
                nc.tensor.matmul(
                    xps[:, ts(t, BL)], sxu[:, t, :], identb64[:],
                    is_transpose=True, start=(t == 0), stop=(t == S - 1),
                )
            xu = encpool.tile([128, S * BL], BF16, tag="xu", name="xu")
            nc.any.tensor_copy(xu[:], xps[:])

            # y x-part write (f32 exact)
            nc.sync.dma_start(y_h[:, ts(b, S), 0:D], xf[:])

            rx = xu[0:D, :]
            ru = xu[D:128, :]

            # L1: h1x = relu(Wx1^T x^T + bx1), h1u likewise (K=64)
            h1xs, h1us = [], []
            for mt in range(2):
                psx = encps.tile([128, NTOK], F32, tag="encps", name="encps_t")
                psu = encps.tile([128, NTOK], F32, tag="encps", name="encps_t")
                for hf in range(2):
                    # x on array rows 0:64, u on rows 64:128 -> concurrent
                    nc.tensor.matmul(
                        psx[:, ts(hf, 512)], wx1b[:, ts(mt, 128)], rx[:, ts(hf, 512)],
                        start=True, stop=True,
                    )
                    nc.tensor.matmul(
                        psu[:, ts(hf, 512)], wu1b[64:128, ts(mt, 128)], ru[:, ts(hf, 512)],
                        start=True, stop=True, tile_position=(64, 0),
                    )
                sbx = actpool.tile([128, NTOK], BF16, tag=f"h1x{mt}", name=f"h1x{mt}_t")
                nc.scalar.activation(sbx[:], psx[:], RELU, bias=bx1v[:, mt : mt + 1])
                h1xs.append(sbx)
                sbu = actpool.tile([128, NTOK], BF16, tag=f"h1u{mt}", name=f"h1u{mt}_t")
                nc.vector.tensor_scalar(
                    sbu[:], psu[:], bu1v[:, mt : mt + 1], 0.0,
                    op0=mybir.AluOpType.add, op1=mybir.AluOpType.max,
                )
                h1us.append(sbu)

            # L2: h2x = relu(Wx2^T h1x + bx2)
            # both mt psums live; loops ordered so consecutive matmuls
            # stream the same moving operand (avoids rhs-stream restarts)
            ps2 = [encps.tile([128, NTOK], F32, tag="encps", name="encps_t") for _ in range(2)]
            for lt in range(2):
                for hf in range(2):
                    for mt in range(2):
                        nc.tensor.matmul(
                            ps2[mt][:, ts(hf, 512)], wx2b[lt][:, ts(mt, 128)],
                            h1xs[lt][:, ts(hf, 512)],
                            start=(lt == 0), stop=(lt == 1),
                        )
            h2xs = []
            for mt in range(2):
                sb = actpool.tile([128, NTOK], BF16, tag=f"h2x{mt}", name=f"h2x{mt}_t")
                if mt == 0:
                    nc.scalar.activation(sb[:], ps2[mt][:], RELU, bias=bx2v[:, mt : mt + 1])
                else:
                    nc.vector.tensor_scalar(
                        sb[:], ps2[mt][:], bx2v[:, mt : mt + 1], 0.0,
                        op0=mybir.AluOpType.add, op1=mybir.AluOpType.max,
                    )
                h2xs.append(sb)

            # Bu = u WB[:64] + h1u Wt (K=64 part row-packed at (64,0)),
            # c = Bu + bK -> cz (f32r); rhs-reuse ordering across mt
            psb = [encps.tile([128, NTOK], F32, tag="encps", name="encps_t") for _ in range(2)]
            for hf in range(2):
                for mt in range(2):
                    nc.tensor.matmul(
                        psb[mt][:, ts(hf, 512)], wbub[64:128, ts(mt, 128)],
                        ru[:, ts(hf, 512)],
                        start=True, stop=False, tile_position=(64, 0),
                    )
            for lt in range(2):
                for hf in range(2):
                    for mt in range(2):
                        nc.tensor.matmul(
                            psb[mt][:, ts(hf, 512)], wtb[lt][:, ts(mt, 128)],
                            h1us[lt][:, ts(hf, 512)],
                            start=False, stop=(lt == 1),
                        )
            for mt in range(2):
                czt = get_cz(g, mt)
                nc.vector.tensor_scalar_add(
                    czt[:, :, big, :],
                    psb[mt][:].rearrange("p (a c) -> p a c", a=S),
                    bkv[:, mt : mt + 1],
                )

            # g (natural layout) = h2x @ Wx3: 8 M-tiles in 2 psum tiles
            gps = [encps.tile([128, NTOK], F32, tag="encps", name="gps_t") for _ in range(2)]
            for mt8 in range(8):
                out = gps[mt8 // 4][:, (mt8 % 4) * 256 : (mt8 % 4) * 256 + G]
                for lt in range(2):
                    nc.tensor.matmul(
                        out, h2xs[lt][:, ts(mt8, 128)], wx3b[lt][:],
                        start=(lt == 0), stop=(lt == 1),
                    )
            gs = actpool.tile([128, 8, G], F32, tag="gs", name="gs_t")
            for half in range(2):
                nc.any.tensor_copy(
                    gs[:, ts(half, 4), :],
                    gps[half][:].rearrange("p (m x) -> p m x", m=4)[:, :, 0:G],
                )
            for j2 in range(2):
                nc.sync.dma_start(yv_g[b, j2][:, :, D:L], gs[ts(j2, 64), :, :])

            # y0 (t = 0): assemble y_start[0] transposed, fp32 x-part
            if b == 0:
                y0a = sps.tile([128, BL], F32, tag="sps", name="y0a_t")
                nc.tensor.matmul(
                    y0a[0:D, :], xf[:, 0, :], ident[0:D, 0:D],
                    is_transpose=True, start=True, stop=True,
                )
                for lt in range(2):
                    nc.tensor.matmul(
                        y0a[64:128, :], wx3b[lt][:, 0:64], h2xs[lt][:, 0:BL],
                        start=(lt == 0), stop=(lt == 1), tile_position=(0, 64),
                    )
                nc.any.tensor_copy(get_ys(0, 0)[:, 0, :], y0a[:])
                y0b = sps.tile([128, BL], F32, tag="sps", name="y0b_t")
                for lt in range(2):
                    nc.tensor.matmul(
                        y0b[:], wx3b[lt][:, 64:192], h2xs[lt][:, 0:BL],
                        start=(lt == 0), stop=(lt == 1),
                    )
                nc.any.tensor_copy(get_ys(0, 1)[:, 0, :], y0b[:])

        # ------------------------------------------------------------------
        # Phase 1: batched local scans (per group)
        # ------------------------------------------------------------------
        def phase1(g):
            czt = [get_cz(g, lt) for lt in range(2)]
            for j in range(1, S):
                zprev = [czt[lt][:, j - 1, :, :].rearrange("p a c -> p (a c)") for lt in range(2)]
                ps = sps.tile([128, 512], F32, tag="sps", name="p1ps_t")
                for l1t in range(2):
                    for l2t in range(2):
                        nc.tensor.matmul(
                            ps[:, ts(l2t, GB * BL)],
                            pr[(1, l1t)][:, ts(l2t, 128)], zprev[l1t],
                            start=(l1t == 0 and l2t == 0),
                            stop=(l1t == 1 and l2t == 1),
                        )
                for l2t in range(2):
                    nc.vector.tensor_add(
                        czt[l2t][:, j, :, :],
                        ps[:, ts(l2t, GB * BL)].rearrange("p (b c) -> p b c", b=GB),
                        czt[l2t][:, j, :, :],
                    )

        # ------------------------------------------------------------------
        # Phase 2: block-level scan (serial, 4 steps per group)
        # ------------------------------------------------------------------
        def phase2(g):
            for nb in range(GB):
                b = g * GB + nb
                if b == NB - 1:
                    break
                ng, nnb = (g, nb + 1) if nb + 1 < GB else (g + 1, 0)
                ps = sps.tile([128, 2 * BL], F32, tag="sps", name="p2ps_t")
                for l1t in range(2):
                    for lt in range(2):
                        nc.tensor.matmul(
                            ps[:, ts(lt, BL)],
                            pr[(S, l1t)][:, ts(lt, 128)], get_ys(g, l1t)[:, nb, :],
                            start=(l1t == 0 and lt == 0),
                            stop=(l1t == 1 and lt == 1),
                        )
                for lt in range(2):
                    nc.vector.tensor_add(
                        get_ys(ng, lt)[:, nnb, :], ps[:, ts(lt, BL)],
                        get_cz(g, lt)[:, S - 1, nb, :].bitcast(F32),
                    )

        # ------------------------------------------------------------------
        # Phase 3: fix-up, natural-layout output
        # ------------------------------------------------------------------
        def phase3(g):
            for j in range(S):
                ysb = yppool.tile([128, 2, L], F32, tag="ysb", name="ysb_t")
                ps = sps.tile([128, 2 * L], F32, tag="sps", name="p3ps_t")
                if j > 0:
                    for l1t in range(2):
                        for mt in range(2):
                            nc.tensor.matmul(
                                ps[:, ts(mt, L)],
                                get_ys(g, l1t)[:, ts(mt, 2), :].rearrange("p a c -> p (a c)"),
                                pr[(j, l1t)][:],
                                start=(mt == 0 and l1t == 0), stop=False,
                            )
                    for mt in range(2):
                        for lt in range(2):
                            nc.tensor.matmul(
                                ps[:, mt * L + lt * 128 : mt * L + lt * 128 + 128],
                                get_cz(g, lt)[:, j - 1, ts(mt, 2), :].rearrange("p a c -> p (a c)").bitcast(F32),
                                ident[:],
                                is_transpose=True, start=False,
                                stop=(mt == 1 and lt == 1),
                            )
                else:
                    for mt in range(2):
                        for lt in range(2):
                            nc.tensor.matmul(
                                ps[:, mt * L + lt * 128 : mt * L + lt * 128 + 128],
                                get_ys(g, lt)[:, ts(mt, 2), :].rearrange("p a c -> p (a c)").bitcast(F32),
                                ident[:],
                                is_transpose=True, start=(mt == 0 and lt == 0),
                                stop=(mt == 1 and lt == 1),
                            )
                nc.any.tensor_copy(ysb[:], ps[:].rearrange("p (m x) -> p m x", m=2))
                for nb2 in range(2):
                    nc.sync.dma_start(ypv[g, j][nb2], ysb[ts(nb2, 64), :, :])

        # ------------------------------------------------------------------
        # Emit
        # ------------------------------------------------------------------
        for g in range(NG):
            for big in range(GB):
                encoder_chunk(g * GB + big)
            phase1(g)
            phase2(g)
            phase3(g)

    nc.compile()
    return nc


_NC = None


def _get_nc():
    global _NC
    if _NC is None:
        _NC = _build()
    return _NC


def kernel(**inputs):
    nc = _get_nc()
    wnames = [
        "Wx1", "bx1", "Wx2", "bx2", "Wx3", "Wu1", "bu1", "Wu2", "WB", "WK", "bK",
    ]
    weights = {k: np.ascontiguousarray(np.asarray(inputs[k], dtype=np.float32)) for k in wnames}
    x = np.asarray(inputs["x"], dtype=np.float32)
    u = np.asarray(inputs["u"], dtype=np.float32)
    in_maps = []
    for c in range(NCORES):
        m = dict(weights)
        m["x"] = np.ascontiguousarray(x[c * BL : (c + 1) * BL])
        m["u"] = np.ascontiguousarray(u[c * BL : (c + 1) * BL])
        in_maps.append(m)
    res = run_bass_kernel_spmd(nc, in_maps, core_ids=list(range(NCORES)))
    y = np.concatenate([r["y"] for r in res.results], axis=0)
    y_pred = np.concatenate([r["y_pred"] for r in res.results], axis=0)
    return (y, y_pred)


# revision 21
# speedup vs baseline: 1.2062x; 1.0035x over previous
"""Trainium2 Bass kernel for nn_DEINA: encoder + Koopman linear recurrence.

Self-contained: shards the batch (512 trajectories) over 8 NeuronCores
(64 trajectories each), runs a fused encoder + blocked-scan recurrence
per core, and gathers the full outputs.

Math (per trajectory, T=256 steps, D=64, H=256, G=192, L=256):
    g  = relu(x Wx1 + bx1); g = relu(g Wx2 + bx2); g = g Wx3
    y  = [x, g]                                  (output 1)
    v  = relu(u Wu1 + bu1) Wu2;  uu = [u, v];  Bu = uu WB
    y_pred[0] = y[0];  y_pred[t+1] = y_pred[t] K + bK + Bu[t]   (output 2)

v3 = v1 blocked-scan recurrence + two structural wins:
  - u-path folded: Bu = u WB[:64] + relu(u Wu1 + bu1) (Wu2 WB[64:]),
    with Wt = Wu2 WB[64:] precomputed on device. No uu materialization.
  - K-power chain in f32r (4x faster than fp32 on the PE).

The time recurrence is parallelized as a blocked scan with S=16:
    phase 1: per block b, z[b,0]=0; z[b,j+1] = z[b,j] K + c[b*S+j]
             (batched over all blocks -> wide matmuls)
    phase 2: y_start[b+1] = y_start[b] K^S + z[b,S]   (16 tiny serial steps)
    phase 3: y_pred[b*S+j] = y_start[b] K^j + z[b,j]  (parallel over b,j;
             emitted directly in natural [token, L] layout)

Precision: encoder in bf16 (fp32 accumulate), recurrence in f32r
(~13-bit mantissa, full PE rate) with K powers chained in f32r.
"""

import numpy as np

import concourse.bacc as bacc
import concourse.bass as bass
import concourse.tile as tile
from concourse import mybir
from concourse.bass import ts
from concourse.bass_utils import run_bass_kernel_spmd
from concourse.masks import make_identity

F32 = mybir.dt.float32
F32R = mybir.dt.float32r
BF16 = mybir.dt.bfloat16
RELU = mybir.ActivationFunctionType.Relu

NCORES = 8
BL = 64  # trajectories per core
T = 256
D = 64
H = 256
G = 192
L = 256
S = 16  # scan block size (= time steps per chunk)
NB = 16  # number of blocks
NG = 4  # block groups
GB = 4  # blocks per group
NTOK = S * BL  # tokens per block wave


def _build():
    nc = bacc.Bacc("TRN2", target_bir_lowering=False)

    x_h = nc.dram_tensor("x", [BL, T, D], F32, kind="ExternalInput")
    u_h = nc.dram_tensor("u", [BL, T, D], F32, kind="ExternalInput")
    wx1_h = nc.dram_tensor("Wx1", [D, H], F32, kind="ExternalInput")
    bx1_h = nc.dram_tensor("bx1", [H], F32, kind="ExternalInput")
    wx2_h = nc.dram_tensor("Wx2", [H, H], F32, kind="ExternalInput")
    bx2_h = nc.dram_tensor("bx2", [H], F32, kind="ExternalInput")
    wx3_h = nc.dram_tensor("Wx3", [H, G], F32, kind="ExternalInput")
    wu1_h = nc.dram_tensor("Wu1", [D, H], F32, kind="ExternalInput")
    bu1_h = nc.dram_tensor("bu1", [H], F32, kind="ExternalInput")
    wu2_h = nc.dram_tensor("Wu2", [H, G], F32, kind="ExternalInput")
    wb_h = nc.dram_tensor("WB", [L, L], F32, kind="ExternalInput")
    wk_h = nc.dram_tensor("WK", [L, L], F32, kind="ExternalInput")
    bk_h = nc.dram_tensor("bK", [L], F32, kind="ExternalInput")
    y_h = nc.dram_tensor("y", [BL, T, L], F32, kind="ExternalOutput")
    yp_h = nc.dram_tensor("y_pred", [BL, T, L], F32, kind="ExternalOutput")

    with tile.TileContext(nc) as tc, tile.ExitStack() as ctx:
        wpool = ctx.enter_context(tc.tile_pool(name="w", bufs=1))
        encpool = ctx.enter_context(tc.tile_pool(name="enc", bufs=3))
        actpool = ctx.enter_context(tc.tile_pool(name="act", bufs=2))
        czpool = ctx.enter_context(tc.tile_pool(name="cz", bufs=2))
        yspool = ctx.enter_context(tc.tile_pool(name="ys", bufs=2))
        yppool = ctx.enter_context(tc.tile_pool(name="ypd", bufs=3))
        pwpool = ctx.enter_context(tc.tile_pool(name="pw", bufs=2))
        encps = ctx.enter_context(tc.tile_pool(name="encps", bufs=2, space="PSUM"))
        tpps = ctx.enter_context(tc.tile_pool(name="tpps", bufs=1, space="PSUM"))
        sps = ctx.enter_context(tc.tile_pool(name="sps", bufs=3, space="PSUM"))

        # ------------------------------------------------------------------
        # Head fast-path: input slabs for blocks 0-1 lead the sync queue
        # ------------------------------------------------------------------
        fastin = {}
        for b in range(2):
            xf0 = encpool.tile([BL, S, D], F32, tag="xf", name="xf")
            nc.sync.dma_start(xf0[:], x_h[:, ts(b, S), :])
            uf0 = encpool.tile([BL, S, D], F32, tag="uf", name=f"uf{b}")
            nc.sync.dma_start(uf0[:], u_h[:, ts(b, S), :])
            fastin[b] = (xf0, uf0)

        # ------------------------------------------------------------------
        # Weights / constants
        # ------------------------------------------------------------------
        def load_f32(ap, shape, name):
            t = wpool.tile(shape, F32, tag=name, name=name)
            nc.sync.dma_start(t[:], ap)
            return t

        def to_bf16(src, name):
            t = wpool.tile(list(src.shape), BF16, tag=name, name=name)
            nc.vector.tensor_copy(t[:], src[:])
            return t

        # L1 weights: wx1 lives on partitions 0:64, wu1 on 64:128
        wx1f = wpool.tile([D, H], F32, tag="wx1f")
        nc.sync.dma_start(wx1f[:], wx1_h[:, :])
        wx1b = to_bf16(wx1f, "wx1b")
        wu1f = wpool.tile([128, H], F32, tag="wu1f")
        nc.sync.dma_start(wu1f[64:128, :], wu1_h[:, :])
        wu1b = wpool.tile([128, H], BF16, tag="wu1b")
        nc.vector.tensor_copy(wu1b[64:128, :], wu1f[64:128, :])

        # WB rows 0:64 on partitions 64:128 (lhsT for the K=64 Bu part)
        wbuf = wpool.tile([128, L], F32, tag="wbuf")
        nc.sync.dma_start(wbuf[64:128, :], wb_h[0:64, :])
        wbub = wpool.tile([128, L], BF16, tag="wbub")
        nc.vector.tensor_copy(wbub[64:128, :], wbuf[64:128, :])

        wx2b, wx3b = [], []
        for lt in range(2):
            wx2b.append(to_bf16(load_f32(wx2_h.ap()[ts(lt, 128), :], [128, H], f"wx2f{lt}"), f"wx2b{lt}"))
            wx3b.append(to_bf16(load_f32(wx3_h.ap()[ts(lt, 128), :], [128, G], f"wx3f{lt}"), f"wx3b{lt}"))

        # for Wt = Wu2 @ WB[64:]
        wu2f = [load_f32(wu2_h.ap()[ts(lt, 128), :], [128, G], f"wu2f{lt}") for lt in range(2)]
        wblf0 = load_f32(wb_h.ap()[64:192, :], [128, L], "wblf0")
        wblf1 = load_f32(wb_h.ap()[192:256, :], [64, L], "wblf1")

        # biases as per-partition scalars: col j holds b[j*128 + p]
        def load_bias(h, name):
            t = wpool.tile([128, 2], F32, tag=name, name=name)
            nc.sync.dma_start(t[:], h.rearrange("(t p) -> p t", p=128))
            return t

        bx1v = load_bias(bx1_h, "bx1v")
        bx2v = load_bias(bx2_h, "bx2v")
        bu1v = load_bias(bu1_h, "bu1v")
        bkv = load_bias(bk_h, "bkv")

        # K tiles (f32) and identities
        kf = [load_f32(wk_h.ap()[ts(lt, 128), :], [128, L], f"kf{lt}") for lt in range(2)]
        ident = wpool.tile([128, 128], F32, tag="ident")
        make_identity(nc, ident[:])
        # identity64 on partitions 0:64 (for the input PE transposes)
        identb64 = wpool.tile([64, 64], BF16, tag="identb64")
        nc.gpsimd.dma_start(identb64[:], ident[0:64, 0:64])

        # K^T tiles (for the power chain): kT[b][p, a] = K[a, b*128+p]
        kT = [wpool.tile([128, L], F32, tag=f"kT{lt}", name=f"kT{lt}") for lt in range(2)]
        for a in range(2):
            for b in range(2):
                pst = sps.tile([128, 128], F32, tag="sps", name="pstT_t")
                nc.tensor.transpose(pst[:], kf[a][:, ts(b, 128)], ident[:])
                nc.scalar.copy(kT[b][:, ts(a, 128)], pst[:])

        # Wu2^T: g-tile0 [128, 256h], g-tile1 [64, 256h]
        wu2T0 = wpool.tile([128, H], F32, tag="wu2T0")
        wu2T1 = wpool.tile([64, H], F32, tag="wu2T1")
        for lt in range(2):
            pst = sps.tile([128, 128], F32, tag="sps", name="wu2T_t")
            nc.tensor.transpose(pst[:], wu2f[lt][:, 0:128], ident[:])
            nc.scalar.copy(wu2T0[:, ts(lt, 128)], pst[:])
            pst = sps.tile([128, 128], F32, tag="sps", name="wu2T_t")
            nc.tensor.transpose(pst[0:64, :], wu2f[lt][:, 128:192], ident[:])
            nc.scalar.copy(wu2T1[:, ts(lt, 128)], pst[0:64, :])

        # Wt = Wu2 @ WB[64:]  -> [H, L] bf16, 2 h-tiles
        wtb = []
        for ht in range(2):
            ps = sps.tile([128, L], F32, tag="sps", name="wt_t")
            nc.tensor.matmul(ps[:], wu2T0[:, ts(ht, 128)], wblf0[:], start=True, stop=False)
            nc.tensor.matmul(ps[:], wu2T1[:, ts(ht, 128)], wblf1[:], start=False, stop=True)
            t = wpool.tile([128, L], BF16, tag=f"wtb{ht}", name=f"wtb{ht}")
            nc.any.tensor_copy(t[:], ps[:])
            wtb.append(t)

        # K powers P_j = K^j (natural layout), j = 1..16, f32r chain.
        kTr = []
        for bt in range(2):
            t = wpool.tile([128, L], F32R, tag=f"kTr{bt}")
            nc.vector.tensor_copy(t[:], kT[bt][:])
            kTr.append(t)
        pr = {}  # (j, lt) -> f32r tile
        for lt in range(2):
            pr[(1, lt)] = wpool.tile([128, L], F32R, tag=f"pr1_{lt}", name=f"pr1_{lt}")
            nc.vector.tensor_copy(pr[(1, lt)][:], kf[lt][:])
        pf_prev = [pr[(1, lt)] for lt in range(2)]
        for j in range(2, S + 1):
            psts = [sps.tile([128, L], F32, tag="sps", name="pstP_t") for _ in range(2)]
            for bt in range(2):
                for rt in range(2):
                    nc.tensor.matmul(
                        psts[rt][:],
                        kTr[bt][:, ts(rt, 128)],
                        pf_prev[bt][:],
                        start=(bt == 0),
                        stop=(bt == 1),
                    )
            pf_cur = []
            for rt in range(2):
                pr[(j, rt)] = wpool.tile([128, L], F32R, tag=f"pr{j}_{rt}", name=f"pr{j}_{rt}")
                nc.any.tensor_copy(pr[(j, rt)][:], psts[rt][:])
                pf_cur.append(pr[(j, rt)])
            pf_prev = pf_cur

        # ------------------------------------------------------------------
        # Views for strided HBM I/O
        # ------------------------------------------------------------------
        # g-part of y: rows (j2, traj), free (mt, l); t = b*16 + mt*2 + j2
        yv_g = y_h.rearrange("traj (b mt j2) l -> b j2 traj mt l", b=NB, mt=8, j2=2)
        # y_pred: rows (nb2, traj), free (mt, l) for fixed (group, j)
        ypv = yp_h.rearrange(
            "traj (g mt nb2 j) l -> g j nb2 traj mt l", g=NG, mt=2, nb2=2, j=S
        )

        cz = {}  # (group, lt) -> [128, S, GB, BL] f32r tile
        ys = {}  # (group, lt) -> [128, GB, BL] f32r tile

        def get_cz(g, lt):
            if (g, lt) not in cz:
                cz[(g, lt)] = czpool.tile([128, S, GB, BL], F32R, tag=f"cz{lt}", name=f"cz{g}_{lt}")
            return cz[(g, lt)]

        def get_ys(g, lt):
            if (g, lt) not in ys:
                ys[(g, lt)] = yspool.tile([128, GB, BL], F32R, tag=f"ys{lt}", name=f"ys{g}_{lt}")
            return ys[(g, lt)]

        # ------------------------------------------------------------------
        # Encoder chunk: one block b (16 time steps x 64 trajectories),
        # processed as one 1024-token wave
        # ------------------------------------------------------------------
        def encoder_chunk(b):
            g = b // GB
            big = b % GB
            # natural f32 x slab [traj, t, d] (feeds y x-part and, for b=0, y0)
            # blocks 0-1 use the idle sync queue + vector casts (fast head);
            # later blocks use gpsimd cast-DMAs (keeps vector queue free)
            if b < 2:
                xf, uf = fastin[b]
                sxu = encpool.tile([BL, S, 2 * D], BF16, tag="sxu", name="sxu")
                nc.vector.tensor_copy(sxu[:, :, 0:D], xf[:])
                nc.vector.tensor_copy(sxu[:, :, D : 2 * D], uf[:])
            else:
                xf = encpool.tile([BL, S, D], F32, tag="xf", name="xf")
                nc.gpsimd.dma_start(xf[:], x_h[:, ts(b, S), :])
                sxu = encpool.tile([BL, S, 2 * D], BF16, tag="sxu", name="sxu")
                nc.gpsimd.dma_start(sxu[:, :, 0:D], x_h[:, ts(b, S), :])
                nc.gpsimd.dma_start(sxu[:, :, D : 2 * D], u_h[:, ts(b, S), :])

            # PE-transpose the 16 [64,128] t-slabs into one psum bank:
            # partitions (x-d | u-d), cols (t, traj)
            xps = tpps.tile([128, S * BL], BF16, tag="tpps", name="tpps_t")
            for t in range(S):
                nc.tensor.matmul(
                    xps[:, ts(t, BL)], sxu[:, t, :], identb64[:],
                    is_transpose=True, start=(t == 0), stop=(t == S - 1),
                )
            xu = encpool.tile([128, S * BL], BF16, tag="xu", name="xu")
            nc.any.tensor_copy(xu[:], xps[:])

            # y x-part write (f32 exact)
            nc.sync.dma_start(y_h[:, ts(b, S), 0:D], xf[:])

            rx = xu[0:D, :]
            ru = xu[D:128, :]

            # L1: h1x = relu(Wx1^T x^T + bx1), h1u likewise (K=64)
            h1xs, h1us = [], []
            for mt in range(2):
                psx = encps.tile([128, NTOK], F32, tag="encps", name="encps_t")
                psu = encps.tile([128, NTOK], F32, tag="encps", name="encps_t")
                for hf in range(2):
                    # x on array rows 0:64, u on rows 64:128 -> concurrent
                    nc.tensor.matmul(
                        psx[:, ts(hf, 512)], wx1b[:, ts(mt, 128)], rx[:, ts(hf, 512)],
                        start=True, stop=True,
                    )
                    nc.tensor.matmul(
                        psu[:, ts(hf, 512)], wu1b[64:128, ts(mt, 128)], ru[:, ts(hf, 512)],
                        start=True, stop=True, tile_position=(64, 0),
                    )
                sbx = actpool.tile([128, NTOK], BF16, tag=f"h1x{mt}", name=f"h1x{mt}_t")
                nc.scalar.activation(sbx[:], psx[:], RELU, bias=bx1v[:, mt : mt + 1])
                h1xs.append(sbx)
                sbu = actpool.tile([128, NTOK], BF16, tag=f"h1u{mt}", name=f"h1u{mt}_t")
                if mt == 0:
                    nc.scalar.activation(sbu[:], psu[:], RELU, bias=bu1v[:, mt : mt + 1])
                else:
                    nc.vector.tensor_scalar(
                        sbu[:], psu[:], bu1v[:, mt : mt + 1], 0.0,
                        op0=mybir.AluOpType.add, op1=mybir.AluOpType.max,
                    )
                h1us.append(sbu)

            # L2: h2x = relu(Wx2^T h1x + bx2)
            # both mt psums live; loops ordered so consecutive matmuls
            # stream the same moving operand (avoids rhs-stream restarts)
            ps2 = [encps.tile([128, NTOK], F32, tag="encps", name="encps_t") for _ in range(2)]
            for lt in range(2):
                for hf in range(2):
                    for mt in range(2):
                        nc.tensor.matmul(
                            ps2[mt][:, ts(hf, 512)], wx2b[lt][:, ts(mt, 128)],
                            h1xs[lt][:, ts(hf, 512)],
                            start=(lt == 0), stop=(lt == 1),
                        )
            h2xs = []
            for mt in range(2):
                sb = actpool.tile([128, NTOK], BF16, tag=f"h2x{mt}", name=f"h2x{mt}_t")
                if mt == 0:
                    nc.scalar.activation(sb[:], ps2[mt][:], RELU, bias=bx2v[:, mt : mt + 1])
                else:
                    nc.vector.tensor_scalar(
                        sb[:], ps2[mt][:], bx2v[:, mt : mt + 1], 0.0,
                        op0=mybir.AluOpType.add, op1=mybir.AluOpType.max,
                    )
                h2xs.append(sb)

            # Bu = u WB[:64] + h1u Wt (K=64 part row-packed at (64,0)),
            # c = Bu + bK -> cz (f32r); rhs-reuse ordering across mt
            psb = [encps.tile([128, NTOK], F32, tag="encps", name="encps_t") for _ in range(2)]
            for hf in range(2):
                for mt in range(2):
                    nc.tensor.matmul(
                        psb[mt][:, ts(hf, 512)], wbub[64:128, ts(mt, 128)],
                        ru[:, ts(hf, 512)],
                        start=True, stop=False, tile_position=(64, 0),
                    )
            for lt in range(2):
                for hf in range(2):
                    for mt in range(2):
                        nc.tensor.matmul(
                            psb[mt][:, ts(hf, 512)], wtb[lt][:, ts(mt, 128)],
                            h1us[lt][:, ts(hf, 512)],
                            start=False, stop=(lt == 1),
                        )
            for mt in range(2):
                czt = get_cz(g, mt)
                nc.vector.tensor_scalar_add(
                    czt[:, :, big, :],
                    psb[mt][:].rearrange("p (a c) -> p a c", a=S),
                    bkv[:, mt : mt + 1],
                )

            # g (natural layout) = h2x @ Wx3: 8 M-tiles in 2 psum tiles
            gps = [encps.tile([128, NTOK], F32, tag="encps", name="gps_t") for _ in range(2)]
            for mt8 in range(8):
                out = gps[mt8 // 4][:, (mt8 % 4) * 256 : (mt8 % 4) * 256 + G]
                for lt in range(2):
                    nc.tensor.matmul(
                        out, h2xs[lt][:, ts(mt8, 128)], wx3b[lt][:],
                        start=(lt == 0), stop=(lt == 1),
                    )
            gs = actpool.tile([128, 8, G], F32, tag="gs", name="gs_t")
            for half in range(2):
                nc.any.tensor_copy(
                    gs[:, ts(half, 4), :],
                    gps[half][:].rearrange("p (m x) -> p m x", m=4)[:, :, 0:G],
                )
            for j2 in range(2):
                nc.sync.dma_start(yv_g[b, j2][:, :, D:L], gs[ts(j2, 64), :, :])

            # y0 (t = 0): assemble y_start[0] transposed, fp32 x-part
            if b == 0:
                y0a = sps.tile([128, BL], F32, tag="sps", name="y0a_t")
                nc.tensor.matmul(
                    y0a[0:D, :], xf[:, 0, :], ident[0:D, 0:D],
                    is_transpose=True, start=True, stop=True,
                )
                for lt in range(2):
                    nc.tensor.matmul(
                        y0a[64:128, :], wx3b[lt][:, 0:64], h2xs[lt][:, 0:BL],
                        start=(lt == 0), stop=(lt == 1), tile_position=(0, 64),
                    )
                nc.any.tensor_copy(get_ys(0, 0)[:, 0, :], y0a[:])
                y0b = sps.tile([128, BL], F32, tag="sps", name="y0b_t")
                for lt in range(2):
                    nc.tensor.matmul(
                        y0b[:], wx3b[lt][:, 64:192], h2xs[lt][:, 0:BL],
                        start=(lt == 0), stop=(lt == 1),
                    )
                nc.any.tensor_copy(get_ys(0, 1)[:, 0, :], y0b[:])

        # ------------------------------------------------------------------
        # Phase 1: batched local scans (per group)
        # ------------------------------------------------------------------
        def phase1(g):
            czt = [get_cz(g, lt) for lt in range(2)]
            for j in range(1, S):
                zprev = [czt[lt][:, j - 1, :, :].rearrange("p a c -> p (a c)") for lt in range(2)]
                ps = sps.tile([128, 512], F32, tag="sps", name="p1ps_t")
                for l1t in range(2):
                    for l2t in range(2):
                        nc.tensor.matmul(
                            ps[:, ts(l2t, GB * BL)],
                            pr[(1, l1t)][:, ts(l2t, 128)], zprev[l1t],
                            start=(l1t == 0 and l2t == 0),
                            stop=(l1t == 1 and l2t == 1),
                        )
                for l2t in range(2):
                    nc.vector.tensor_add(
                        czt[l2t][:, j, :, :],
                        ps[:, ts(l2t, GB * BL)].rearrange("p (b c) -> p b c", b=GB),
                        czt[l2t][:, j, :, :],
                    )

        # ------------------------------------------------------------------
        # Phase 2: block-level scan (serial, 4 steps per group)
        # ------------------------------------------------------------------
        def phase2(g):
            for nb in range(GB):
                b = g * GB + nb
                if b == NB - 1:
                    break
                ng, nnb = (g, nb + 1) if nb + 1 < GB else (g + 1, 0)
                ps = sps.tile([128, 2 * BL], F32, tag="sps", name="p2ps_t")
                for l1t in range(2):
                    for lt in range(2):
                        nc.tensor.matmul(
                            ps[:, ts(lt, BL)],
                            pr[(S, l1t)][:, ts(lt, 128)], get_ys(g, l1t)[:, nb, :],
                            start=(l1t == 0 and lt == 0),
                            stop=(l1t == 1 and lt == 1),
                        )
                for lt in range(2):
                    nc.vector.tensor_add(
                        get_ys(ng, lt)[:, nnb, :], ps[:, ts(lt, BL)],
                        get_cz(g, lt)[:, S - 1, nb, :].bitcast(F32),
                    )

        # ------------------------------------------------------------------
        # Phase 3: fix-up, natural-layout output
        # ------------------------------------------------------------------
        def phase3(g):
            for j in range(S):
                ysb = yppool.tile([128, 2, L], F32, tag="ysb", name="ysb_t")
                ps = sps.tile([128, 2 * L], F32, tag="sps", name="p3ps_t")
                if j > 0:
                    for l1t in range(2):
                        for mt in range(2):
                            nc.tensor.matmul(
                                ps[:, ts(mt, L)],
                                get_ys(g, l1t)[:, ts(mt, 2), :].rearrange("p a c -> p (a c)"),
                                pr[(j, l1t)][:],
                                start=(mt == 0 and l1t == 0), stop=False,
                            )
                    for mt in range(2):
                        for lt in range(2):
                            nc.tensor.matmul(
                                ps[:, mt * L + lt * 128 : mt * L + lt * 128 + 128],
                                get_cz(g, lt)[:, j - 1, ts(mt, 2), :].rearrange("p a c -> p (a c)").bitcast(F32),
                                ident[:],
                                is_transpose=True, start=False,
                                stop=(mt == 1 and lt == 1),
                            )
                else:
                    for mt in range(2):
                        for lt in range(2):
                            nc.tensor.matmul(
                                ps[:, mt * L + lt * 128 : mt * L + lt * 128 + 128],
                                get_ys(g, lt)[:, ts(mt, 2), :].rearrange("p a c -> p (a c)").bitcast(F32),
                                ident[:],
                                is_transpose=True, start=(mt == 0 and lt == 0),
                                stop=(mt == 1 and lt == 1),
                            )
                nc.scalar.copy(ysb[:, 0, :], ps[:, 0:L])
                nc.vector.tensor_copy(ysb[:, 1, :], ps[:, L : 2 * L])
                for nb2 in range(2):
                    nc.sync.dma_start(ypv[g, j][nb2], ysb[ts(nb2, 64), :, :])

        # ------------------------------------------------------------------
        # Emit
        # ------------------------------------------------------------------
        for g in range(NG):
            for big in range(GB):
                encoder_chunk(g * GB + big)
            phase1(g)
            phase2(g)
            phase3(g)

    nc.compile()
    return nc


_NC = None


def _get_nc():
    global _NC
    if _NC is None:
        _NC = _build()
    return _NC


def kernel(**inputs):
    nc = _get_nc()
    wnames = [
        "Wx1", "bx1", "Wx2", "bx2", "Wx3", "Wu1", "bu1", "Wu2", "WB", "WK", "bK",
    ]
    weights = {k: np.ascontiguousarray(np.asarray(inputs[k], dtype=np.float32)) for k in wnames}
    x = np.asarray(inputs["x"], dtype=np.float32)
    u = np.asarray(inputs["u"], dtype=np.float32)
    in_maps = []
    for c in range(NCORES):
        m = dict(weights)
        m["x"] = np.ascontiguousarray(x[c * BL : (c + 1) * BL])
        m["u"] = np.ascontiguousarray(u[c * BL : (c + 1) * BL])
        in_maps.append(m)
    res = run_bass_kernel_spmd(nc, in_maps, core_ids=list(range(NCORES)))
    y = np.concatenate([r["y"] for r in res.results], axis=0)
    y_pred = np.concatenate([r["y_pred"] for r in res.results], axis=0)
    return (y, y_pred)
